# revision 4
# baseline (speedup 1.0000x reference)
"""Bass/Trainium2 kernel for a 2-layer bidirectional LSTM (CustomBiLSTM).

Strategy: data-parallel over batch across 8 NeuronCores (B=64 -> 8 per core).
Per core, each layer runs its forward and backward recurrent chains in
LOCKSTEP, fused into a single instruction stream: every engine op covers
both chains' data at once ([128, 2*...] tiles).  This halves the per-step
instruction count, eliminates cross-chain queue collisions, and keeps one
clean serial dependency chain per step:

    PE (8 recurrent matmuls) -> ACT sigmoid (all gates, both chains)
      -> DVE m2/m1/c (cell update) -> ACT tanh -> DVE h-mult -> PE ...

Gate pre-activations for a window of CH=8 steps x 2 chains live in one
PSUM bank as [128, 2*4*CH*8]; the input projections (Wih @ x) plus bias
(rank-1 matmul against a ones row) are precomputed off the critical path
at reduced scheduler priority.

The backward chain's inputs/outputs are stored STEP-indexed (time
reversed).  The host supplies a time-reversed copy of x (xTr) so layer-1
precompute uses only positive strides; during layer 1, idle Pool-engine
copies maintain time-indexed/reversed views of h1 for layer-2 precompute.
The host un-reverses the backward half of the output.

The g-gate weights are pre-scaled by 2 on the host so a single Sigmoid
activation covers all 4 gates (tanh(z) = 2*sigmoid(2z) - 1); the affine
fix-up is fused into the DVE cell-state update.
"""

import numpy as np
import ml_dtypes

try:
    import concourse.bass as bass
except ImportError:
    import sys
    sys.path.insert(0, "/opt/trn_rl_repo")
    import concourse.bass as bass

import concourse.bacc as bacc
import concourse.tile as tile
from concourse import mybir
from concourse.bass_utils import run_bass_kernel_spmd

F32 = mybir.dt.float32
BF16 = mybir.dt.bfloat16
AF = mybir.ActivationFunctionType
ALU = mybir.AluOpType
BF16_NP = ml_dtypes.bfloat16

H = 128          # hidden dim
D = 128          # input dim
B = 64           # global batch
T = 1024         # sequence length
NCORES = 8
BL = B // NCORES  # per-core batch = 8
G = 4            # gates (i, f, g, o)
NCH = 2          # chains per layer (fw=0, bw=1), lockstep-fused
CH = 512 // (NCH * G * BL)  # window steps per PSUM bank (=8)


def build_program(t_len=T):
    nw = t_len // CH
    nc = bacc.Bacc("TRN2", target_bir_lowering=False, debug=False)
    TB = t_len * BL

    # ---- DRAM I/O ----
    xT_d = nc.dram_tensor("xT", [D, TB], BF16, kind="ExternalInput")
    xTr_d = nc.dram_tensor("xTr", [D, TB], BF16, kind="ExternalInput")
    whh_d, wih_d, bias_d = {}, {}, {}
    for lay in (1, 2):
        for dirn in ("a", "b"):
            cell = f"{dirn}{lay}"
            whh_d[cell] = nc.dram_tensor(f"whhT_{cell}", [H, G * H], BF16,
                                         kind="ExternalInput")
            bias_d[cell] = nc.dram_tensor(f"bias_{cell}", [1, G * H], BF16,
                                          kind="ExternalInput")
            nchunk = 1 if lay == 1 else 2
            wih_d[cell] = [
                nc.dram_tensor(f"wihT_{cell}_{q}", [H, G * H], BF16,
                               kind="ExternalInput")
                for q in range(nchunk)
            ]
    o2_d = nc.dram_tensor("o2", [H, NCH * TB], BF16, kind="ExternalOutput")

    with tile.TileContext(nc) as tc:
        with tc.tile_pool(name="const", bufs=1) as const, \
             tc.tile_pool(name="ps", bufs=1, space="PSUM") as psp, \
             tc.tile_pool(name="work", bufs=8) as work:

            # ---- persistent SBUF ----
            xT = const.tile([D, TB], BF16, tag="xT")
            xTr = const.tile([D, TB], BF16, tag="xTr")
            ndma = 8
            chunk = TB // ndma
            for src_t, dst_t in ((xT_d, xT), (xTr_d, xTr)):
                for i in range(ndma):
                    nc.sync.dma_start(out=dst_t[:, i * chunk:(i + 1) * chunk],
                                      in_=src_t.ap()[:, i * chunk:(i + 1) * chunk])

            whh_s, wih_s, bias_s = {}, {}, {}
            for cell in whh_d:
                whh_s[cell] = const.tile([H, G * H], BF16, name=f"whh_{cell}")
                nc.sync.dma_start(out=whh_s[cell][:, :], in_=whh_d[cell].ap()[:, :])
                bias_s[cell] = const.tile([1, G * H], BF16, name=f"bias_{cell}")
                nc.sync.dma_start(out=bias_s[cell][:, :], in_=bias_d[cell].ap()[:, :])
                wih_s[cell] = []
                for q, dd in enumerate(wih_d[cell]):
                    wt = const.tile([H, G * H], BF16, name=f"wih_{cell}_{q}")
                    nc.sync.dma_start(out=wt[:, :], in_=dd.ap()[:, :])
                    wih_s[cell].append(wt)

            ones_row = const.tile([1, CH * BL], BF16, tag="ones_row")
            nc.vector.memset(ones_row[:, :], 1.0)

            # h storage: fused per layer, chain-major halves, STEP-indexed
            # (bw half holds time T-1-k at step k).
            h1 = const.tile([H, NCH * TB], BF16, name="h1")
            h2 = const.tile([H, NCH * TB], BF16, name="h2")
            # time-indexed copy of bw half / step-reversed copy of fw half
            # (maintained by Pool during layer 1; read by layer-2 precompute)
            h1bt = const.tile([H, TB], BF16, name="h1bt")
            h1ar = const.tile([H, TB], BF16, name="h1ar")

            # 2 psum window banks (parity ping-pong), both chains per bank
            pb = [psp.tile([H, NCH * G * CH * BL], F32, name=f"ps{p}")
                  for p in (0, 1)]

            W = CH * BL  # columns per (chain, gate) region in a bank

            for lay in (1, 2):
                cells = (f"a{lay}", f"b{lay}")
                hout = h1 if lay == 1 else h2
                if lay == 1:
                    rhs_src = {0: [xT], 1: [xTr]}
                else:
                    # chain 0 (fw, step=time): [h1 fw half, h1bt]
                    # chain 1 (bw, step k -> time T-1-k): [h1ar, h1 bw half]
                    rhs_src = {0: [h1, h1bt], 1: [h1ar, h1]}
                    rhs_off = {0: [0, 0], 1: [0, TB]}

                def precompute(j):
                    """Fill psum bank j%2 with Wih@x + bias for both chains."""
                    ps = pb[j % 2]
                    with tc.high_priority(offset=-1_000_000):
                        first = True
                        for ci in range(NCH):
                            srcs = rhs_src[ci]
                            for g in range(G):
                                reg = (ci * G + g) * W
                                for q, src in enumerate(srcs):
                                    off = j * W if lay == 1 else \
                                        rhs_off[ci][q] + j * W
                                    nc.tensor.matmul(
                                        ps[:, reg:reg + W],
                                        wih_s[cells[ci]][q][:, g * H:(g + 1) * H],
                                        src[:, off:off + W],
                                        start=first, stop=False)
                                    first = False
                        for ci in range(NCH):
                            for g in range(G):
                                reg = (ci * G + g) * W
                                nc.tensor.matmul(
                                    ps[:, reg:reg + W],
                                    bias_s[cells[ci]][:, g * H:(g + 1) * H],
                                    ones_row[:, :],
                                    start=False,
                                    stop=(ci == NCH - 1 and g == G - 1))

                precompute(0)
                c_prev = None
                for k in range(t_len):
                    j, slot = k // CH, k % CH
                    ps = pb[j % 2]
                    if k > 0:
                        # recurrent matmuls accumulate onto precomputed psum
                        for ci in range(NCH):
                            hprev = hout[:, ci * TB + (k - 1) * BL:
                                         ci * TB + k * BL]
                            for g in range(G):
                                off = (ci * G + g) * W + slot * BL
                                nc.tensor.matmul(
                                    ps[:, off:off + BL],
                                    whh_s[cells[ci]][:, g * H:(g + 1) * H],
                                    hprev, start=False, stop=False,
                                    skip_group_check=True)
                    s = work.tile([H, NCH * G * BL], F32, name="s")
                    ps_view = ps[:, :].rearrange(
                        "p (c g t b) -> p c g t b",
                        c=NCH, g=G, t=CH)[:, :, :, slot, :]
                    s_view = s[:, :].rearrange("p (c g b) -> p c g b",
                                               c=NCH, g=G)
                    nc.scalar.activation(s_view, ps_view, AF.Sigmoid)

                    sc = s[:, :].rearrange("p (c x) -> p c x", c=NCH)
                    si, sf = sc[:, :, 0:BL], sc[:, :, BL:2 * BL]
                    s2g, so = sc[:, :, 2 * BL:3 * BL], sc[:, :, 3 * BL:4 * BL]
                    m2 = work.tile([H, NCH * BL], F32, name="m2")
                    m2v = m2[:, :].rearrange("p (c b) -> p c b", c=NCH)
                    # m2 = (sigma(2g)-0.5)*sigma(i) = 0.5 * i_gate * tanh(g)
                    nc.vector.scalar_tensor_tensor(m2v, s2g, 0.5, si,
                                                   ALU.subtract, ALU.mult)
                    c = work.tile([H, NCH * BL], F32, name="c")
                    cv = c[:, :].rearrange("p (c b) -> p c b", c=NCH)
                    if k > 0:
                        m1 = work.tile([H, NCH * BL], F32, name="m1")
                        m1v = m1[:, :].rearrange("p (c b) -> p c b", c=NCH)
                        nc.vector.tensor_tensor(m1v, sf, c_prev[:, :].rearrange(
                            "p (c b) -> p c b", c=NCH), ALU.mult)
                        nc.vector.scalar_tensor_tensor(cv, m2v, 2.0, m1v,
                                                       ALU.mult, ALU.add)
                    else:
                        nc.vector.tensor_scalar_mul(c, m2, 2.0)
                    c_prev = c
                    th = work.tile([H, NCH * BL], F32, name="th")
                    nc.scalar.activation(th, c, AF.Tanh)
                    hview = hout[:, :].rearrange("p (c t b) -> p c t b",
                                                 c=NCH, b=BL)[:, :, k, :]
                    thv = th[:, :].rearrange("p (c b) -> p c b", c=NCH)
                    nc.vector.tensor_tensor(hview, so, thv, ALU.mult)

                    if lay == 1:
                        # maintain layer-2 precompute layouts on idle Pool
                        tau = t_len - 1 - k
                        nc.gpsimd.tensor_copy(
                            h1bt[:, tau * BL:(tau + 1) * BL],
                            h1[:, TB + k * BL:TB + (k + 1) * BL])
                        nc.gpsimd.tensor_copy(
                            h1ar[:, tau * BL:(tau + 1) * BL],
                            h1[:, k * BL:(k + 1) * BL])

                    if slot == CH // 2 - 1 and j + 1 < nw:
                        precompute(j + 1)

                    if lay == 2:
                        ock = t_len // 8
                        if (k + 1) % ock == 0:
                            lo = (k + 1 - ock) * BL
                            hi = (k + 1) * BL
                            for ci in range(NCH):
                                nc.sync.dma_start(
                                    out=o2_d.ap()[:, ci * TB + lo:ci * TB + hi],
                                    in_=h2[:, ci * TB + lo:ci * TB + hi])

    nc.compile()
    return nc


def _prep_weights(Wih, Whh, bih, bhh):
    """Host-side weight massaging: transpose, gate-scale (g-gate x2), bf16."""
    gscale = np.array([1.0, 1.0, 2.0, 1.0], np.float32)
    fourh, ind = Wih.shape
    wihT = np.ascontiguousarray(Wih.T).astype(np.float32)      # [in, 4H]
    whhT = np.ascontiguousarray(Whh.T).astype(np.float32)      # [H, 4H]
    bias = (bih + bhh).astype(np.float32)                      # [4H]
    for g in range(G):
        sl = slice(g * H, (g + 1) * H)
        wihT[:, sl] *= gscale[g]
        whhT[:, sl] *= gscale[g]
        bias[sl] *= gscale[g]
    nq = ind // H
    wih_chunks = [np.ascontiguousarray(wihT[q * H:(q + 1) * H]).astype(BF16_NP)
                  for q in range(nq)]
    bias_row = bias.reshape(1, G * H).astype(BF16_NP)           # [1, 4H]
    return wih_chunks, whhT.astype(BF16_NP), bias_row


def core_xT(xs, t_len):
    """Per-core input layouts: xs [BL, T, D] -> xT, xTr both [D, T*BL]."""
    fw = xs.transpose(2, 1, 0).reshape(D, t_len * BL)
    bw = xs[:, ::-1, :].transpose(2, 1, 0).reshape(D, t_len * BL)
    return (np.ascontiguousarray(fw).astype(BF16_NP),
            np.ascontiguousarray(bw).astype(BF16_NP))


def core_gather(res_c, t_len):
    """Per-core output: o2 [H, 2*T*BL] (bw half step-indexed) -> [BL,T,2H]."""
    o = np.asarray(res_c["o2"]).astype(np.float32)
    TB = t_len * BL
    out = np.empty((BL, t_len, 2 * H), np.float32)
    fw = o[:, :TB].reshape(H, t_len, BL)
    bw = o[:, TB:].reshape(H, t_len, BL)[:, ::-1, :]  # step k -> time T-1-k
    out[:, :, :H] = fw.transpose(2, 1, 0)
    out[:, :, H:] = bw.transpose(2, 1, 0)
    return out


_PROG_CACHE = {}


def prepare_in_maps(x, kw):
    x = np.asarray(x, np.float32)
    t_len = x.shape[1]
    cells = {"a1": (kw["Wih_fw1"], kw["Whh_fw1"], kw["bih_fw1"], kw["bhh_fw1"]),
             "b1": (kw["Wih_bw1"], kw["Whh_bw1"], kw["bih_bw1"], kw["bhh_bw1"]),
             "a2": (kw["Wih_fw2"], kw["Whh_fw2"], kw["bih_fw2"], kw["bhh_fw2"]),
             "b2": (kw["Wih_bw2"], kw["Whh_bw2"], kw["bih_bw2"], kw["bhh_bw2"])}
    wmaps = {}
    for cell, (Wih, Whh, bih, bhh) in cells.items():
        wih_chunks, whhT, bias_row = _prep_weights(
            np.asarray(Wih, np.float32), np.asarray(Whh, np.float32),
            np.asarray(bih, np.float32), np.asarray(bhh, np.float32))
        wmaps[f"whhT_{cell}"] = whhT
        wmaps[f"bias_{cell}"] = bias_row
        for q, wc in enumerate(wih_chunks):
            wmaps[f"wihT_{cell}_{q}"] = wc

    core_ids = list(range(NCORES))
    in_maps = []
    for c in core_ids:
        fw, bw = core_xT(x[c * BL:(c + 1) * BL], t_len)
        m = {"xT": fw, "xTr": bw}
        m.update(wmaps)
        in_maps.append(m)
    return in_maps, core_ids


def kernel(x, lengths, **kw):
    x = np.asarray(x, np.float32)
    t_len = x.shape[1]
    in_maps, core_ids = prepare_in_maps(x, kw)
    if t_len not in _PROG_CACHE:
        _PROG_CACHE[t_len] = build_program(t_len)
    nc = _PROG_CACHE[t_len]
    return _execute(nc, in_maps, core_ids, t_len)[0]


def _execute(nc, in_maps, core_ids, t_len, **run_kwargs):
    r = run_bass_kernel_spmd(nc, in_maps, core_ids, **run_kwargs)
    out = np.empty((B, t_len, 2 * H), np.float32)
    for c in core_ids:
        out[c * BL:(c + 1) * BL] = core_gather(r.results[c], t_len)
    return out, r


# revision 15
# speedup vs baseline: 1.0754x; 1.0754x over previous
"""Bass/Trainium2 kernel for a 2-layer bidirectional LSTM (CustomBiLSTM).

Strategy: data-parallel over batch across 8 NeuronCores (B=64 -> 8 per core).
Per core, each layer runs its forward and backward recurrent chains in
LOCKSTEP, fused into a single instruction stream: every engine op covers
both chains' data at once ([128, 2*...] tiles), giving one serial
dependency chain per step:

    PE (8 recurrent matmuls) -> ACT sigmoid (all gates, both chains)
      -> DVE m2/m1/c (cell update) -> ACT tanh -> DVE h-mult -> PE ...

Synchronization is HAND-PLACED raw-bass semaphores (no Tile auto-sync):
each instruction carries at most ONE attached semaphore wait, chosen so
engine sequencers never block on standalone EventSemaphore instructions
(decode stays off the critical path) and same-engine ordering rides on
queue order instead of semaphore round-trips.  Ring-buffer reuse (s/m/c/th,
PSUM banks) is proven safe transitively through the single per-step
dependency chain.

Gate pre-activations for a window of CH=8 steps x 2 chains live in one
PSUM bank as [128, 2*4*CH*8]; input projections (Wih @ x) plus bias
(rank-1 matmul against a ones row) are precomputed a half-window ahead,
off the critical path.

The backward chain's inputs/outputs are stored STEP-indexed (time
reversed).  The host supplies a time-reversed copy of x (xTr) so layer-1
precompute uses only positive strides; during layer 1, idle Pool-engine
copies maintain time-indexed/reversed views of h1 for layer-2 precompute.
The host un-reverses the backward half of the output.

The g-gate weights are pre-scaled by 2 on the host so a single Sigmoid
activation covers all 4 gates (tanh(z) = 2*sigmoid(2z) - 1); the affine
fix-up is fused into the DVE cell-state update.
"""

import numpy as np
import ml_dtypes

try:
    import concourse.bass as bass
except ImportError:
    import sys
    sys.path.insert(0, "/opt/trn_rl_repo")
    import concourse.bass as bass

import concourse.bacc as bacc
from concourse import mybir
from concourse.bass_utils import run_bass_kernel_spmd

F32 = mybir.dt.float32
BF16 = mybir.dt.bfloat16
AF = mybir.ActivationFunctionType
ALU = mybir.AluOpType
BF16_NP = ml_dtypes.bfloat16

H = 128          # hidden dim
D = 128          # input dim
B = 64           # global batch
T = 1024         # sequence length
NCORES = 8
BL = B // NCORES  # per-core batch = 8
G = 4            # gates (i, f, g, o)
NCH = 2          # chains per layer (fw=0, bw=1), lockstep-fused
CH = 512 // (NCH * G * BL)  # window steps per PSUM bank (=8)




def build_program(t_len=T):
    nw = t_len // CH
    nc = bacc.Bacc("TRN2", target_bir_lowering=False, debug=False)
    TB = t_len * BL

    # ---- DRAM I/O ----
    xT_d = nc.dram_tensor("xT", [D, TB], BF16, kind="ExternalInput")
    xTr_d = nc.dram_tensor("xTr", [D, TB], BF16, kind="ExternalInput")
    whh_d, wih_d, bias_d = {}, {}, {}
    for lay in (1, 2):
        for dirn in ("a", "b"):
            cell = f"{dirn}{lay}"
            whh_d[cell] = nc.dram_tensor(f"whhT_{cell}", [H, G * H], BF16,
                                         kind="ExternalInput")
            bias_d[cell] = nc.dram_tensor(f"bias_{cell}", [1, G * H], BF16,
                                          kind="ExternalInput")
            nchunk = 1 if lay == 1 else 2
            wih_d[cell] = [
                nc.dram_tensor(f"wihT_{cell}_{q}", [H, G * H], BF16,
                               kind="ExternalInput")
                for q in range(nchunk)
            ]
    o2_d = nc.dram_tensor("o2", [H, NCH * TB], BF16, kind="ExternalOutput")

    # ---- SBUF ----
    xT = nc.alloc_sbuf_tensor("xT_s", [D, TB], BF16)
    xTr = nc.alloc_sbuf_tensor("xTr_s", [D, TB], BF16)
    whh_s, wih_s, bias_s = {}, {}, {}
    for cell in whh_d:
        whh_s[cell] = nc.alloc_sbuf_tensor(f"whh_{cell}", [H, G * H], BF16)
        bias_s[cell] = nc.alloc_sbuf_tensor(f"biasr_{cell}", [1, G * H], BF16)
        wih_s[cell] = [
            nc.alloc_sbuf_tensor(f"wih_{cell}_{q}", [H, G * H], BF16)
            for q in range(len(wih_d[cell]))
        ]
    ones_row = nc.alloc_sbuf_tensor("ones_row", [1, CH * BL], BF16)
    c_init = nc.alloc_sbuf_tensor("c_init", [H, NCH * BL], F32)
    h1 = nc.alloc_sbuf_tensor("h1", [H, NCH * TB], BF16)
    h2 = nc.alloc_sbuf_tensor("h2", [H, NCH * TB], BF16)
    h1bt = nc.alloc_sbuf_tensor("h1bt", [H, TB], BF16)
    h1ar = nc.alloc_sbuf_tensor("h1ar", [H, TB], BF16)
    # work rings (depth 2)
    s_r = [nc.alloc_sbuf_tensor(f"s{i}", [H, NCH * G * BL], F32)
           for i in (0, 1)]
    m2_r = [nc.alloc_sbuf_tensor(f"m2_{i}", [H, NCH * BL], F32)
            for i in (0, 1)]
    m1_r = [nc.alloc_sbuf_tensor(f"m1_{i}", [H, NCH * BL], F32)
            for i in (0, 1)]
    c_r = [nc.alloc_sbuf_tensor(f"c_{i}", [H, NCH * BL], F32) for i in (0, 1)]
    th_r = [nc.alloc_sbuf_tensor(f"th_{i}", [H, NCH * BL], F32)
            for i in (0, 1)]
    # ---- PSUM ----
    pb = [nc.alloc_psum_tensor(f"pbank{p}", [H, NCH * G * CH * BL], F32)
          for p in (0, 1)]

    # ---- semaphores ----
    S_IN = nc.alloc_semaphore("S_IN")
    S_PE = nc.alloc_semaphore("S_PE")
    S_ACT = nc.alloc_semaphore("S_ACT")
    S_DVE = nc.alloc_semaphore("S_DVE")
    S_POOL = nc.alloc_semaphore("S_POOL")
    S_OUT = nc.alloc_semaphore("S_OUT")

    cnt = {"pe": 0, "act": 0, "dve": 0, "pool": 0, "in": 0, "out": 0}

    # ---- input DMAs ----
    ndma = 8
    chunk = TB // ndma
    for src_t, dst_t in ((xT_d, xT), (xTr_d, xTr)):
        for i in range(ndma):
            nc.sync.dma_start(out=dst_t.ap()[:, i * chunk:(i + 1) * chunk],
                              in_=src_t.ap()[:, i * chunk:(i + 1) * chunk]
                              ).then_inc(S_IN, 16)
            cnt["in"] += 16
    for cell in whh_d:
        nc.sync.dma_start(out=whh_s[cell].ap()[:, :],
                          in_=whh_d[cell].ap()[:, :]).then_inc(S_IN, 16)
        nc.sync.dma_start(out=bias_s[cell].ap()[:, :],
                          in_=bias_d[cell].ap()[:, :]).then_inc(S_IN, 16)
        cnt["in"] += 32
        for q, dd in enumerate(wih_d[cell]):
            nc.sync.dma_start(out=wih_s[cell][q].ap()[:, :],
                              in_=dd.ap()[:, :]).then_inc(S_IN, 16)
            cnt["in"] += 16

    nc.vector.memset(ones_row.ap()[:, :], 1.0).then_inc(S_DVE, 1)
    nc.vector.memset(c_init.ap()[:, :], 0.0).then_inc(S_DVE, 1)
    cnt["dve"] += 2
    # gate the Pool engine on c_init being zeroed (m1 of step 0 reads it)
    nc.gpsimd.wait_ge(S_DVE, cnt["dve"])

    # PE prologue gates: inputs + ones_row ready
    nc.tensor.wait_ge(S_IN, cnt["in"])
    nc.tensor.wait_ge(S_DVE, cnt["dve"])

    W = CH * BL  # columns per (chain, gate) region in a bank
    # S_ACT count after the last sigma of each (lay, window) — for bank WAR
    act_after_win = {}

    for lay in (1, 2):
        cells = (f"a{lay}", f"b{lay}")
        hout = h1 if lay == 1 else h2
        if lay == 1:
            rhs_src = {0: [(xT, 0)], 1: [(xTr, 0)]}
        else:
            # chain 0 (fw, step=time): [h1 fw half, h1bt]
            # chain 1 (bw, step k -> time T-1-k): [h1ar, h1 bw half]
            rhs_src = {0: [(h1, 0), (h1bt, 0)], 1: [(h1ar, 0), (h1, TB)]}

        def precompute(j, inc_pe=False):
            """Fill psum bank j%2 with Wih@x + bias for both chains."""
            ps = pb[j % 2].ap()
            war = act_after_win.pop((lay, j), None)
            first = True
            for ci in range(NCH):
                for g in range(G):
                    reg = (ci * G + g) * W
                    for q, (src, base) in enumerate(rhs_src[ci]):
                        mm = nc.tensor.matmul(
                            ps[:, reg:reg + W],
                            wih_s[cells[ci]][q].ap()[:, g * H:(g + 1) * H],
                            src.ap()[:, base + j * W:base + (j + 1) * W],
                            start=first, stop=False)
                        if first and war is not None:
                            mm._wait_ge(S_ACT, war)
                        first = False
            for ci in range(NCH):
                for g in range(G):
                    reg = (ci * G + g) * W
                    mm = nc.tensor.matmul(
                        ps[:, reg:reg + W],
                        bias_s[cells[ci]].ap()[:, g * H:(g + 1) * H],
                        ones_row.ap()[:, :],
                        start=False, stop=(ci == NCH - 1 and g == G - 1))
            if inc_pe:
                mm.then_inc(S_PE, 1)
                cnt["pe"] += 1

        if lay == 2:
            # layer boundary: h1 / h1bt / h1ar fully written
            nc.tensor.wait_ge(S_DVE, cnt["dve"])
            nc.tensor.wait_ge(S_POOL, cnt["pool"])
        precompute(0, inc_pe=True)
        precompute(1)

        h_cnt_prev = None
        c_prev = c_init.ap()[:, :].rearrange("p (c b) -> p c b", c=NCH)
        for k in range(t_len):
            j, slot = k // CH, k % CH
            ps = pb[j % 2].ap()
            if k > 0:
                first = True
                for ci in range(NCH):
                    hprev = hout.ap()[:, ci * TB + (k - 1) * BL:
                                      ci * TB + k * BL]
                    for g in range(G):
                        off = (ci * G + g) * W + slot * BL
                        mm = nc.tensor.matmul(
                            ps[:, off:off + BL],
                            whh_s[cells[ci]].ap()[:, g * H:(g + 1) * H],
                            hprev, start=False, stop=False,
                            skip_group_check=True)
                        if first:
                            mm._wait_ge(S_DVE, h_cnt_prev)
                            first = False
                mm.then_inc(S_PE, 1)
                cnt["pe"] += 1

            s = s_r[k % 2].ap()
            ps_view = ps[:, :].rearrange("p (c g t b) -> p c g t b",
                                         c=NCH, g=G, t=CH)[:, :, :, slot, :]
            s_view = s[:, :].rearrange("p (c g b) -> p c g b", c=NCH, g=G)
            sig = nc.scalar.activation(s_view, ps_view, AF.Sigmoid)
            sig._wait_ge(S_PE, cnt["pe"])
            sig.then_inc(S_ACT, 1)
            cnt["act"] += 1
            if slot == CH - 1:
                act_after_win[(lay, j + 2)] = cnt["act"]

            sc = s[:, :].rearrange("p (c x) -> p c x", c=NCH)
            si, sf = sc[:, :, 0:BL], sc[:, :, BL:2 * BL]
            s2g, so = sc[:, :, 2 * BL:3 * BL], sc[:, :, 3 * BL:4 * BL]
            m2 = m2_r[k % 2].ap()
            m2v = m2[:, :].rearrange("p (c b) -> p c b", c=NCH)
            # m2 = (sigma(2g)-0.5)*sigma(i) = 0.5 * i_gate * tanh(g)
            op = nc.vector.scalar_tensor_tensor(m2v, s2g, 0.5, si,
                                                ALU.subtract, ALU.mult)
            op._wait_ge(S_ACT, cnt["act"])
            # m1 on Pool, concurrent with m2 on DVE; the c-op's S_POOL wait
            # also time-separates it from m2's in-flight SBUF write
            m1 = m1_r[k % 2].ap()
            m1v = m1[:, :].rearrange("p (c b) -> p c b", c=NCH)
            m1op = nc.gpsimd.tensor_tensor(m1v, sf, c_prev, ALU.mult)
            m1op._wait_ge(S_ACT, cnt["act"])
            m1op.then_inc(S_POOL, 1)
            cnt["pool"] += 1
            c = c_r[k % 2].ap()
            cv = c[:, :].rearrange("p (c b) -> p c b", c=NCH)
            op = nc.vector.scalar_tensor_tensor(cv, m2v, 2.0, m1v,
                                                ALU.mult, ALU.add)
            op._wait_ge(S_POOL, cnt["pool"])
            op.then_inc(S_DVE, 1)
            cnt["dve"] += 1
            c_prev = cv
            th = th_r[k % 2].ap()
            th_i = nc.scalar.activation(th[:, :], c[:, :], AF.Tanh)
            th_i._wait_ge(S_DVE, cnt["dve"])
            th_i.then_inc(S_ACT, 1)
            cnt["act"] += 1
            hview = hout.ap()[:, :].rearrange("p (c t b) -> p c t b",
                                              c=NCH, b=BL)[:, :, k, :]
            thv = th[:, :].rearrange("p (c b) -> p c b", c=NCH)
            hop = nc.vector.tensor_tensor(hview, so, thv, ALU.mult)
            hop._wait_ge(S_ACT, cnt["act"])
            hop.then_inc(S_DVE, 1)
            cnt["dve"] += 1
            h_cnt_prev = cnt["dve"]

            if lay == 1:
                tau = t_len - 1 - k
                cp = nc.gpsimd.tensor_copy(
                    h1bt.ap()[:, tau * BL:(tau + 1) * BL],
                    h1.ap()[:, TB + k * BL:TB + (k + 1) * BL])
                cp._wait_ge(S_DVE, cnt["dve"])
                cp.then_inc(S_POOL, 1)
                nc.gpsimd.tensor_copy(
                    h1ar.ap()[:, tau * BL:(tau + 1) * BL],
                    h1.ap()[:, k * BL:(k + 1) * BL]).then_inc(S_POOL, 1)
                cnt["pool"] += 2

            if slot == CH // 2 - 1 and j + 1 < nw:
                precompute(j + 1)

            if lay == 2:
                ock = t_len // 8
                if (k + 1) % ock == 0:
                    lo = (k + 1 - ock) * BL
                    hi = (k + 1) * BL
                    for ci in range(NCH):
                        dma = nc.sync.dma_start(
                            out=o2_d.ap()[:, ci * TB + lo:ci * TB + hi],
                            in_=h2.ap()[:, ci * TB + lo:ci * TB + hi])
                        dma._wait_ge(S_DVE, cnt["dve"])
                        dma.then_inc(S_OUT, 16)
                        cnt["out"] += 16

    nc.sync.wait_ge(S_OUT, cnt["out"])
    nc.all_engine_barrier()
    nc.compile()
    return nc


def _prep_weights(Wih, Whh, bih, bhh):
    """Host-side weight massaging: transpose, gate-scale (g-gate x2), bf16."""
    gscale = np.array([1.0, 1.0, 2.0, 1.0], np.float32)
    fourh, ind = Wih.shape
    wihT = np.ascontiguousarray(Wih.T).astype(np.float32)      # [in, 4H]
    whhT = np.ascontiguousarray(Whh.T).astype(np.float32)      # [H, 4H]
    bias = (bih + bhh).astype(np.float32)                      # [4H]
    for g in range(G):
        sl = slice(g * H, (g + 1) * H)
        wihT[:, sl] *= gscale[g]
        whhT[:, sl] *= gscale[g]
        bias[sl] *= gscale[g]
    nq = ind // H
    wih_chunks = [np.ascontiguousarray(wihT[q * H:(q + 1) * H]).astype(BF16_NP)
                  for q in range(nq)]
    bias_row = bias.reshape(1, G * H).astype(BF16_NP)           # [1, 4H]
    return wih_chunks, whhT.astype(BF16_NP), bias_row


def core_xT(xs, t_len):
    """Per-core input layouts: xs [BL, T, D] -> xT, xTr both [D, T*BL]."""
    fw = xs.transpose(2, 1, 0).reshape(D, t_len * BL)
    bw = xs[:, ::-1, :].transpose(2, 1, 0).reshape(D, t_len * BL)
    return (np.ascontiguousarray(fw).astype(BF16_NP),
            np.ascontiguousarray(bw).astype(BF16_NP))


def core_gather(res_c, t_len):
    """Per-core output: o2 [H, 2*T*BL] (bw half step-indexed) -> [BL,T,2H]."""
    o = np.asarray(res_c["o2"]).astype(np.float32)
    TB = t_len * BL
    out = np.empty((BL, t_len, 2 * H), np.float32)
    fw = o[:, :TB].reshape(H, t_len, BL)
    bw = o[:, TB:].reshape(H, t_len, BL)[:, ::-1, :]  # step k -> time T-1-k
    out[:, :, :H] = fw.transpose(2, 1, 0)
    out[:, :, H:] = bw.transpose(2, 1, 0)
    return out


_PROG_CACHE = {}


def prepare_in_maps(x, kw):
    x = np.asarray(x, np.float32)
    t_len = x.shape[1]
    cells = {"a1": (kw["Wih_fw1"], kw["Whh_fw1"], kw["bih_fw1"], kw["bhh_fw1"]),
             "b1": (kw["Wih_bw1"], kw["Whh_bw1"], kw["bih_bw1"], kw["bhh_bw1"]),
             "a2": (kw["Wih_fw2"], kw["Whh_fw2"], kw["bih_fw2"], kw["bhh_fw2"]),
             "b2": (kw["Wih_bw2"], kw["Whh_bw2"], kw["bih_bw2"], kw["bhh_bw2"])}
    wmaps = {}
    for cell, (Wih, Whh, bih, bhh) in cells.items():
        wih_chunks, whhT, bias_row = _prep_weights(
            np.asarray(Wih, np.float32), np.asarray(Whh, np.float32),
            np.asarray(bih, np.float32), np.asarray(bhh, np.float32))
        wmaps[f"whhT_{cell}"] = whhT
        wmaps[f"bias_{cell}"] = bias_row
        for q, wc in enumerate(wih_chunks):
            wmaps[f"wihT_{cell}_{q}"] = wc

    core_ids = list(range(NCORES))
    in_maps = []
    for c in core_ids:
        fw, bw = core_xT(x[c * BL:(c + 1) * BL], t_len)
        m = {"xT": fw, "xTr": bw}
        m.update(wmaps)
        in_maps.append(m)
    return in_maps, core_ids


def kernel(x, lengths, **kw):
    x = np.asarray(x, np.float32)
    t_len = x.shape[1]
    in_maps, core_ids = prepare_in_maps(x, kw)
    if t_len not in _PROG_CACHE:
        _PROG_CACHE[t_len] = build_program(t_len)
    nc = _PROG_CACHE[t_len]
    return _execute(nc, in_maps, core_ids, t_len)[0]


def _execute(nc, in_maps, core_ids, t_len, **run_kwargs):
    r = run_bass_kernel_spmd(nc, in_maps, core_ids, **run_kwargs)
    out = np.empty((B, t_len, 2 * H), np.float32)
    for c in core_ids:
        out[c * BL:(c + 1) * BL] = core_gather(r.results[c], t_len)
    return out, r


# revision 16
# speedup vs baseline: 1.1251x; 1.0462x over previous
"""Bass/Trainium2 kernel for a 2-layer bidirectional LSTM (CustomBiLSTM).

Strategy: data-parallel over batch across 8 NeuronCores (B=64 -> 8 per core).
Per core, each layer runs its forward (A) and backward (B) recurrent chains
as two INDEPENDENT instruction streams, phase-staggered by construction:
both loops have identical latency, so a constant offset is self-sustaining,
and the smaller per-chain ops ([128, 8..32]) make the serial chain shorter
than a lockstep-fused one.

Per chain and step:
    PE (4 recurrent matmuls) -> ACT sigmoid (4 gates) -> DVE m2 || Pool m1
      -> DVE c -> ACT tanh -> DVE h-mult -> PE ...

Synchronization is HAND-PLACED raw-bass semaphores (no Tile auto-sync):
each instruction carries at most ONE attached semaphore wait so engine
sequencers never block on standalone EventSemaphore instructions, keeping
decode off the critical path.  Same-engine ordering rides on queue order;
the c-op's Pool-semaphore wait also time-separates it from m2's in-flight
same-engine SBUF write.  Ring-buffer reuse (s/m2/m1/c/th, PSUM banks) is
safe transitively through each chain's dependency chain.

Gate pre-activations for a window of CH=16 steps live in one PSUM bank per
chain as [128, 4*CH*8]; input projections (Wih @ x) plus bias (rank-1
matmul against a ones row) are precomputed a half-window ahead, off the
critical path.

The backward chain's inputs/outputs are stored STEP-indexed (time
reversed).  The host supplies a time-reversed copy of x (xTr) so layer-1
precompute uses only positive strides; during layer 1, idle Pool-engine
copies maintain time-indexed/reversed views of h1 for layer-2 precompute.
The host un-reverses the backward half of the output.

The g-gate weights are pre-scaled by 2 on the host so a single Sigmoid
activation covers all 4 gates (tanh(z) = 2*sigmoid(2z) - 1); the affine
fix-up is fused into the DVE cell-state update.
"""

import numpy as np
import ml_dtypes

try:
    import concourse.bass as bass
except ImportError:
    import sys
    sys.path.insert(0, "/opt/trn_rl_repo")
    import concourse.bass as bass

import concourse.bacc as bacc
from concourse import mybir
from concourse.bass_utils import run_bass_kernel_spmd

F32 = mybir.dt.float32
BF16 = mybir.dt.bfloat16
AF = mybir.ActivationFunctionType
ALU = mybir.AluOpType
BF16_NP = ml_dtypes.bfloat16

H = 128          # hidden dim
D = 128          # input dim
B = 64           # global batch
T = 1024         # sequence length
NCORES = 8
BL = B // NCORES  # per-core batch = 8
G = 4            # gates (i, f, g, o)
CH = 512 // (G * BL)  # window steps per PSUM bank per chain (=16)


def build_program(t_len=T):
    nw = t_len // CH
    nc = bacc.Bacc("TRN2", target_bir_lowering=False, debug=False)
    TB = t_len * BL

    # ---- DRAM I/O ----
    xT_d = nc.dram_tensor("xT", [D, TB], BF16, kind="ExternalInput")
    xTr_d = nc.dram_tensor("xTr", [D, TB], BF16, kind="ExternalInput")
    whh_d, wih_d, bias_d = {}, {}, {}
    for lay in (1, 2):
        for dirn in ("a", "b"):
            cell = f"{dirn}{lay}"
            whh_d[cell] = nc.dram_tensor(f"whhT_{cell}", [H, G * H], BF16,
                                         kind="ExternalInput")
            bias_d[cell] = nc.dram_tensor(f"bias_{cell}", [1, G * H], BF16,
                                          kind="ExternalInput")
            nchunk = 1 if lay == 1 else 2
            wih_d[cell] = [
                nc.dram_tensor(f"wihT_{cell}_{q}", [H, G * H], BF16,
                               kind="ExternalInput")
                for q in range(nchunk)
            ]
    o2_d = nc.dram_tensor("o2", [H, 2 * TB], BF16, kind="ExternalOutput")

    # ---- SBUF ----
    xT = nc.alloc_sbuf_tensor("xT_s", [D, TB], BF16)
    xTr = nc.alloc_sbuf_tensor("xTr_s", [D, TB], BF16)
    whh_s, wih_s, bias_s = {}, {}, {}
    for cell in whh_d:
        whh_s[cell] = nc.alloc_sbuf_tensor(f"whh_{cell}", [H, G * H], BF16)
        bias_s[cell] = nc.alloc_sbuf_tensor(f"biasr_{cell}", [1, G * H], BF16)
        wih_s[cell] = [
            nc.alloc_sbuf_tensor(f"wih_{cell}_{q}", [H, G * H], BF16)
            for q in range(len(wih_d[cell]))
        ]
    ones_row = nc.alloc_sbuf_tensor("ones_row", [1, CH * BL], BF16)
    c_init = nc.alloc_sbuf_tensor("c_init", [H, BL], F32)
    # h storage per layer+chain, STEP-indexed (bw: step k -> time T-1-k)
    h_t = {(lay, ci): nc.alloc_sbuf_tensor(f"h{lay}{ci}", [H, TB], BF16)
           for lay in (1, 2) for ci in (0, 1)}
    h1bt = nc.alloc_sbuf_tensor("h1bt", [H, TB], BF16)  # h1 bw, time-indexed
    h1ar = nc.alloc_sbuf_tensor("h1ar", [H, TB], BF16)  # h1 fw, step-reversed
    # work rings (depth 2) per chain
    s_r = {ci: [nc.alloc_sbuf_tensor(f"s{ci}_{i}", [H, G * BL], F32)
                for i in (0, 1)] for ci in (0, 1)}
    m2_r = {ci: [nc.alloc_sbuf_tensor(f"m2{ci}_{i}", [H, BL], F32)
                 for i in (0, 1)] for ci in (0, 1)}
    m1_r = {ci: [nc.alloc_sbuf_tensor(f"m1{ci}_{i}", [H, BL], F32)
                 for i in (0, 1)] for ci in (0, 1)}
    c_r = {ci: [nc.alloc_sbuf_tensor(f"c{ci}_{i}", [H, BL], F32)
                for i in (0, 1)] for ci in (0, 1)}
    th_r = {ci: [nc.alloc_sbuf_tensor(f"th{ci}_{i}", [H, BL], F32)
                 for i in (0, 1)] for ci in (0, 1)}
    # ---- PSUM: 2 banks per chain (window ping-pong) ----
    pb = {ci: [nc.alloc_psum_tensor(f"pb{ci}_{p}", [H, G * CH * BL], F32)
               for p in (0, 1)] for ci in (0, 1)}

    # ---- semaphores ----
    S_IN = nc.alloc_semaphore("S_IN")
    S_PE = nc.alloc_semaphore("S_PE")
    S_ACT = nc.alloc_semaphore("S_ACT")
    S_DVE = nc.alloc_semaphore("S_DVE")
    S_POOL = nc.alloc_semaphore("S_POOL")
    S_OUT = nc.alloc_semaphore("S_OUT")

    cnt = {"pe": 0, "act": 0, "dve": 0, "pool": 0, "in": 0, "out": 0}

    # ---- input DMAs ----
    ndma = 8
    chunk = TB // ndma
    for src_t, dst_t in ((xT_d, xT), (xTr_d, xTr)):
        for i in range(ndma):
            nc.sync.dma_start(out=dst_t.ap()[:, i * chunk:(i + 1) * chunk],
                              in_=src_t.ap()[:, i * chunk:(i + 1) * chunk]
                              ).then_inc(S_IN, 16)
            cnt["in"] += 16
    for cell in whh_d:
        nc.sync.dma_start(out=whh_s[cell].ap()[:, :],
                          in_=whh_d[cell].ap()[:, :]).then_inc(S_IN, 16)
        nc.sync.dma_start(out=bias_s[cell].ap()[:, :],
                          in_=bias_d[cell].ap()[:, :]).then_inc(S_IN, 16)
        cnt["in"] += 32
        for q, dd in enumerate(wih_d[cell]):
            nc.sync.dma_start(out=wih_s[cell][q].ap()[:, :],
                              in_=dd.ap()[:, :]).then_inc(S_IN, 16)
            cnt["in"] += 16

    nc.vector.memset(ones_row.ap()[:, :], 1.0).then_inc(S_DVE, 1)
    nc.vector.memset(c_init.ap()[:, :], 0.0).then_inc(S_DVE, 1)
    cnt["dve"] += 2
    # gate Pool on c_init zero (m1 of step 0 reads it)
    nc.gpsimd.wait_ge(S_DVE, cnt["dve"])

    # PE prologue gates: inputs + ones_row ready
    nc.tensor.wait_ge(S_IN, cnt["in"])
    nc.tensor.wait_ge(S_DVE, cnt["dve"])

    W = CH * BL  # columns per gate region in a bank
    act_after_win = {}   # (lay, ci, window) -> S_ACT count after last sigma

    for lay in (1, 2):
        cells = (f"a{lay}", f"b{lay}")
        if lay == 1:
            rhs_src = {0: [(xT, 0)], 1: [(xTr, 0)]}
        else:
            rhs_src = {0: [(h_t[(1, 0)], 0), (h1bt, 0)],
                       1: [(h1ar, 0), (h_t[(1, 1)], 0)]}

        def precompute(ci, j, inc_pe=False):
            ps = pb[ci][j % 2].ap()
            war = act_after_win.pop((lay, ci, j), None)
            first = True
            for g in range(G):
                reg = g * W
                for q, (src, base) in enumerate(rhs_src[ci]):
                    mm = nc.tensor.matmul(
                        ps[:, reg:reg + W],
                        wih_s[cells[ci]][q].ap()[:, g * H:(g + 1) * H],
                        src.ap()[:, base + j * W:base + (j + 1) * W],
                        start=first, stop=False)
                    if first and war is not None:
                        mm._wait_ge(S_ACT, war)
                    first = False
            for g in range(G):
                mm = nc.tensor.matmul(
                    ps[:, g * W:g * W + W],
                    bias_s[cells[ci]].ap()[:, g * H:(g + 1) * H],
                    ones_row.ap()[:, :],
                    start=False, stop=(g == G - 1))
            if inc_pe:
                mm.then_inc(S_PE, 1)
                cnt["pe"] += 1
            return cnt["pe"]

        if lay == 2:
            # layer boundary: h1 / h1bt / h1ar fully written
            nc.tensor.wait_ge(S_DVE, cnt["dve"])
            nc.tensor.wait_ge(S_POOL, cnt["pool"])
        pe_cnt = {}
        for ci in (0, 1):
            precompute(ci, 0, inc_pe=True)
            pe_cnt[ci] = cnt["pe"]
            precompute(ci, 1)

        h_cnt = {0: None, 1: None}
        c_cnt = {}
        c_prev = {0: c_init.ap()[:, :], 1: c_init.ap()[:, :]}
        for k in range(t_len):
            j, slot = k // CH, k % CH
            # --- PE: recurrence matmuls, chain A then B ---
            if k > 0:
                for ci in (0, 1):
                    ps = pb[ci][j % 2].ap()
                    hprev = h_t[(lay, ci)].ap()[:, (k - 1) * BL:k * BL]
                    first = True
                    for g in range(G):
                        off = g * W + slot * BL
                        mm = nc.tensor.matmul(
                            ps[:, off:off + BL],
                            whh_s[cells[ci]].ap()[:, g * H:(g + 1) * H],
                            hprev, start=False, stop=False,
                            skip_group_check=True)
                        if first:
                            mm._wait_ge(S_DVE, h_cnt[ci])
                            first = False
                    mm.then_inc(S_PE, 1)
                    cnt["pe"] += 1
                    pe_cnt[ci] = cnt["pe"]
            # --- ACT: sigmas (A then B) ---
            sig_cnt = {}
            for ci in (0, 1):
                ps = pb[ci][j % 2].ap()
                s = s_r[ci][k % 2].ap()
                ps_view = ps[:, :].rearrange(
                    "p (g t b) -> p g t b", g=G, t=CH)[:, :, slot, :]
                s_view = s[:, :].rearrange("p (g b) -> p g b", g=G)
                sig = nc.scalar.activation(s_view, ps_view, AF.Sigmoid)
                sig._wait_ge(S_PE, pe_cnt[ci])
                sig.then_inc(S_ACT, 1)
                cnt["act"] += 1
                sig_cnt[ci] = cnt["act"]
                if slot == CH - 1:
                    act_after_win[(lay, ci, j + 2)] = cnt["act"]
            # --- DVE m2 + Pool m1, then DVE c (A then B) ---
            m1_cnt = {}
            for ci in (0, 1):
                s = s_r[ci][k % 2].ap()
                si, sf = s[:, 0:BL], s[:, BL:2 * BL]
                s2g = s[:, 2 * BL:3 * BL]
                m2 = m2_r[ci][k % 2].ap()
                op = nc.vector.scalar_tensor_tensor(
                    m2[:, :], s2g, 0.5, si, ALU.subtract, ALU.mult)
                op._wait_ge(S_ACT, sig_cnt[ci])
                m1 = m1_r[ci][k % 2].ap()
                m1op = nc.gpsimd.tensor_tensor(m1[:, :], sf, c_prev[ci],
                                               ALU.mult)
                m1op._wait_ge(S_ACT, sig_cnt[ci])
                m1op.then_inc(S_POOL, 1)
                cnt["pool"] += 1
                c = c_r[ci][k % 2].ap()
                cop = nc.vector.scalar_tensor_tensor(
                    c[:, :], m2[:, :], 2.0, m1[:, :], ALU.mult, ALU.add)
                cop._wait_ge(S_POOL, cnt["pool"])
                cop.then_inc(S_DVE, 1)
                cnt["dve"] += 1
                c_cnt[ci] = cnt["dve"]
                c_prev[ci] = c[:, :]
            # --- ACT tanh (A then B) ---
            th_cnt = {}
            for ci in (0, 1):
                th = th_r[ci][k % 2].ap()
                ti = nc.scalar.activation(th[:, :], c_r[ci][k % 2].ap()[:, :],
                                          AF.Tanh)
                ti._wait_ge(S_DVE, c_cnt[ci])
                ti.then_inc(S_ACT, 1)
                cnt["act"] += 1
                th_cnt[ci] = cnt["act"]
            # --- DVE h-mult (A then B) ---
            for ci in (0, 1):
                s = s_r[ci][k % 2].ap()
                so = s[:, 3 * BL:4 * BL]
                hop = nc.vector.tensor_tensor(
                    h_t[(lay, ci)].ap()[:, k * BL:(k + 1) * BL],
                    so, th_r[ci][k % 2].ap()[:, :], ALU.mult)
                hop._wait_ge(S_ACT, th_cnt[ci])
                hop.then_inc(S_DVE, 1)
                cnt["dve"] += 1
                h_cnt[ci] = cnt["dve"]
            # --- Pool copies for layer-2 precompute layouts ---
            if lay == 1:
                tau = t_len - 1 - k
                cp1 = nc.gpsimd.tensor_copy(
                    h1bt.ap()[:, tau * BL:(tau + 1) * BL],
                    h_t[(1, 1)].ap()[:, k * BL:(k + 1) * BL])
                cp1._wait_ge(S_DVE, h_cnt[1])
                cp1.then_inc(S_POOL, 1)
                cp2 = nc.gpsimd.tensor_copy(
                    h1ar.ap()[:, tau * BL:(tau + 1) * BL],
                    h_t[(1, 0)].ap()[:, k * BL:(k + 1) * BL])
                cp2._wait_ge(S_DVE, h_cnt[0])
                cp2.then_inc(S_POOL, 1)
                cnt["pool"] += 2
            # --- precompute next window ---
            if slot == CH // 2 - 1 and j + 1 < nw:
                for ci in (0, 1):
                    precompute(ci, j + 1)
            # --- stream output ---
            if lay == 2:
                ock = t_len // 8
                if (k + 1) % ock == 0:
                    lo = (k + 1 - ock) * BL
                    hi = (k + 1) * BL
                    for ci in (0, 1):
                        dma = nc.sync.dma_start(
                            out=o2_d.ap()[:, ci * TB + lo:ci * TB + hi],
                            in_=h_t[(2, ci)].ap()[:, lo:hi])
                        dma._wait_ge(S_DVE, h_cnt[ci])
                        dma.then_inc(S_OUT, 16)
                        cnt["out"] += 16

    nc.sync.wait_ge(S_OUT, cnt["out"])
    nc.all_engine_barrier()
    nc.compile()
    return nc


def _prep_weights(Wih, Whh, bih, bhh):
    """Host-side weight massaging: transpose, gate-scale (g-gate x2), bf16."""
    gscale = np.array([1.0, 1.0, 2.0, 1.0], np.float32)
    fourh, ind = Wih.shape
    wihT = np.ascontiguousarray(Wih.T).astype(np.float32)      # [in, 4H]
    whhT = np.ascontiguousarray(Whh.T).astype(np.float32)      # [H, 4H]
    bias = (bih + bhh).astype(np.float32)                      # [4H]
    for g in range(G):
        sl = slice(g * H, (g + 1) * H)
        wihT[:, sl] *= gscale[g]
        whhT[:, sl] *= gscale[g]
        bias[sl] *= gscale[g]
    nq = ind // H
    wih_chunks = [np.ascontiguousarray(wihT[q * H:(q + 1) * H]).astype(BF16_NP)
                  for q in range(nq)]
    bias_row = bias.reshape(1, G * H).astype(BF16_NP)           # [1, 4H]
    return wih_chunks, whhT.astype(BF16_NP), bias_row


def core_xT(xs, t_len):
    """Per-core input layouts: xs [BL, T, D] -> xT, xTr both [D, T*BL]."""
    fw = xs.transpose(2, 1, 0).reshape(D, t_len * BL)
    bw = xs[:, ::-1, :].transpose(2, 1, 0).reshape(D, t_len * BL)
    return (np.ascontiguousarray(fw).astype(BF16_NP),
            np.ascontiguousarray(bw).astype(BF16_NP))


def core_gather(res_c, t_len):
    """Per-core output: o2 [H, 2*T*BL] (bw half step-indexed) -> [BL,T,2H]."""
    o = np.asarray(res_c["o2"]).astype(np.float32)
    TB = t_len * BL
    out = np.empty((BL, t_len, 2 * H), np.float32)
    fw = o[:, :TB].reshape(H, t_len, BL)
    bw = o[:, TB:].reshape(H, t_len, BL)[:, ::-1, :]  # step k -> time T-1-k
    out[:, :, :H] = fw.transpose(2, 1, 0)
    out[:, :, H:] = bw.transpose(2, 1, 0)
    return out


_PROG_CACHE = {}


def prepare_in_maps(x, kw):
    x = np.asarray(x, np.float32)
    t_len = x.shape[1]
    cells = {"a1": (kw["Wih_fw1"], kw["Whh_fw1"], kw["bih_fw1"], kw["bhh_fw1"]),
             "b1": (kw["Wih_bw1"], kw["Whh_bw1"], kw["bih_bw1"], kw["bhh_bw1"]),
             "a2": (kw["Wih_fw2"], kw["Whh_fw2"], kw["bih_fw2"], kw["bhh_fw2"]),
             "b2": (kw["Wih_bw2"], kw["Whh_bw2"], kw["bih_bw2"], kw["bhh_bw2"])}
    wmaps = {}
    for cell, (Wih, Whh, bih, bhh) in cells.items():
        wih_chunks, whhT, bias_row = _prep_weights(
            np.asarray(Wih, np.float32), np.asarray(Whh, np.float32),
            np.asarray(bih, np.float32), np.asarray(bhh, np.float32))
        wmaps[f"whhT_{cell}"] = whhT
        wmaps[f"bias_{cell}"] = bias_row
        for q, wc in enumerate(wih_chunks):
            wmaps[f"wihT_{cell}_{q}"] = wc

    core_ids = list(range(NCORES))
    in_maps = []
    for c in core_ids:
        fw, bw = core_xT(x[c * BL:(c + 1) * BL], t_len)
        m = {"xT": fw, "xTr": bw}
        m.update(wmaps)
        in_maps.append(m)
    return in_maps, core_ids


def kernel(x, lengths, **kw):
    x = np.asarray(x, np.float32)
    t_len = x.shape[1]
    in_maps, core_ids = prepare_in_maps(x, kw)
    if t_len not in _PROG_CACHE:
        _PROG_CACHE[t_len] = build_program(t_len)
    nc = _PROG_CACHE[t_len]
    return _execute(nc, in_maps, core_ids, t_len)[0]


def _execute(nc, in_maps, core_ids, t_len, **run_kwargs):
    r = run_bass_kernel_spmd(nc, in_maps, core_ids, **run_kwargs)
    out = np.empty((B, t_len, 2 * H), np.float32)
    for c in core_ids:
        out[c * BL:(c + 1) * BL] = core_gather(r.results[c], t_len)
    return out, r


# revision 23
# speedup vs baseline: 1.1281x; 1.0026x over previous
"""Bass/Trainium2 kernel for a 2-layer bidirectional LSTM (CustomBiLSTM).

Strategy: data-parallel over batch across 8 NeuronCores (B=64 -> 8 per core).
Per core, each layer runs its forward (A) and backward (B) recurrent chains
as two INDEPENDENT instruction streams, phase-staggered by construction:
both loops have identical latency, so a constant offset is self-sustaining,
and the smaller per-chain ops ([128, 8..32]) make the serial chain shorter
than a lockstep-fused one.

Per chain and step:
    PE (4 recurrent matmuls) -> ACT sigmoid (4 gates) -> DVE m2 || Pool m1
      -> DVE c -> ACT tanh -> DVE h-mult -> PE ...

Synchronization is HAND-PLACED raw-bass semaphores (no Tile auto-sync):
each instruction carries at most ONE attached semaphore wait so engine
sequencers never block on standalone EventSemaphore instructions, keeping
decode off the critical path.  Same-engine ordering rides on queue order;
the c-op's Pool-semaphore wait also time-separates it from m2's in-flight
same-engine SBUF write.  Ring-buffer reuse (s/m2/m1/c/th, PSUM banks) is
safe transitively through each chain's dependency chain.

Gate pre-activations for a window of CH=16 steps live in one PSUM bank per
chain as [128, 4*CH*8]; input projections (Wih @ x) plus bias (rank-1
matmul against a ones row) are precomputed a half-window ahead, off the
critical path.

The backward chain's inputs/outputs are stored STEP-indexed (time
reversed).  The host supplies a time-reversed copy of x (xTr) so layer-1
precompute uses only positive strides; during layer 1, idle Pool-engine
copies maintain time-indexed/reversed views of h1 for layer-2 precompute.
The host un-reverses the backward half of the output.

The g-gate weights are pre-scaled by 2 on the host so a single Sigmoid
activation covers all 4 gates (tanh(z) = 2*sigmoid(2z) - 1); the affine
fix-up is fused into the DVE cell-state update.
"""

import numpy as np
import ml_dtypes

try:
    import concourse.bass as bass
except ImportError:
    import sys
    sys.path.insert(0, "/opt/trn_rl_repo")
    import concourse.bass as bass

import concourse.bacc as bacc
from concourse import mybir
from concourse.bass_utils import run_bass_kernel_spmd

F32 = mybir.dt.float32
BF16 = mybir.dt.bfloat16
AF = mybir.ActivationFunctionType
ALU = mybir.AluOpType
BF16_NP = ml_dtypes.bfloat16

H = 128          # hidden dim
D = 128          # input dim
B = 64           # global batch
T = 1024         # sequence length
NCORES = 8
BL = B // NCORES  # per-core batch = 8
G = 4            # gates (i, f, g, o)
CH = 512 // (G * BL)  # window steps per PSUM bank per chain (=16)


def build_program(t_len=T):
    nw = t_len // CH
    nc = bacc.Bacc("TRN2", target_bir_lowering=False, debug=False)
    TB = t_len * BL

    # ---- DRAM I/O ----
    xT_d = nc.dram_tensor("xT", [D, TB], BF16, kind="ExternalInput")
    xTr_d = nc.dram_tensor("xTr", [D, TB], BF16, kind="ExternalInput")
    whh_d, wih_d, bias_d = {}, {}, {}
    for lay in (1, 2):
        for dirn in ("a", "b"):
            cell = f"{dirn}{lay}"
            whh_d[cell] = nc.dram_tensor(f"whhT_{cell}", [H, G * H], BF16,
                                         kind="ExternalInput")
            bias_d[cell] = nc.dram_tensor(f"bias_{cell}", [1, G * H], BF16,
                                          kind="ExternalInput")
            nchunk = 1 if lay == 1 else 2
            wih_d[cell] = [
                nc.dram_tensor(f"wihT_{cell}_{q}", [H, G * H], BF16,
                               kind="ExternalInput")
                for q in range(nchunk)
            ]
    o2_d = nc.dram_tensor("o2", [H, 2 * TB], BF16, kind="ExternalOutput")

    # ---- SBUF ----
    xT = nc.alloc_sbuf_tensor("xT_s", [D, TB], BF16)
    xTr = nc.alloc_sbuf_tensor("xTr_s", [D, TB], BF16)
    whh_s, wih_s, bias_s = {}, {}, {}
    for cell in whh_d:
        whh_s[cell] = nc.alloc_sbuf_tensor(f"whh_{cell}", [H, G * H], BF16)
        bias_s[cell] = nc.alloc_sbuf_tensor(f"biasr_{cell}", [1, G * H], BF16)
        wih_s[cell] = [
            nc.alloc_sbuf_tensor(f"wih_{cell}_{q}", [H, G * H], BF16)
            for q in range(len(wih_d[cell]))
        ]
    ones_row = nc.alloc_sbuf_tensor("ones_row", [1, CH * BL], BF16)
    c_init = nc.alloc_sbuf_tensor("c_init", [H, BL], F32)
    # h storage per layer+chain, STEP-indexed (bw: step k -> time T-1-k)
    h_t = {(lay, ci): nc.alloc_sbuf_tensor(f"h{lay}{ci}", [H, TB], BF16)
           for lay in (1, 2) for ci in (0, 1)}
    h1bt = nc.alloc_sbuf_tensor("h1bt", [H, TB], BF16)  # h1 bw, time-indexed
    h1ar = nc.alloc_sbuf_tensor("h1ar", [H, TB], BF16)  # h1 fw, step-reversed
    # work rings (depth 2) per chain
    s_r = {ci: [nc.alloc_sbuf_tensor(f"s{ci}_{i}", [H, G * BL], F32)
                for i in (0, 1)] for ci in (0, 1)}
    m2_r = {ci: [nc.alloc_sbuf_tensor(f"m2{ci}_{i}", [H, BL], F32)
                 for i in (0, 1)] for ci in (0, 1)}
    m1_r = {ci: [nc.alloc_sbuf_tensor(f"m1{ci}_{i}", [H, BL], F32)
                 for i in (0, 1)] for ci in (0, 1)}
    c_r = {ci: [nc.alloc_sbuf_tensor(f"c{ci}_{i}", [H, BL], F32)
                for i in (0, 1)] for ci in (0, 1)}
    th_r = {ci: [nc.alloc_sbuf_tensor(f"th{ci}_{i}", [H, BL], F32)
                 for i in (0, 1)] for ci in (0, 1)}
    # ---- PSUM: 2 banks per chain (window ping-pong) ----
    pb = {ci: [nc.alloc_psum_tensor(f"pb{ci}_{p}", [H, G * CH * BL], F32)
               for p in (0, 1)] for ci in (0, 1)}

    # ---- semaphores ----
    S_IN = nc.alloc_semaphore("S_IN")
    S_PE = nc.alloc_semaphore("S_PE")
    S_ACT = nc.alloc_semaphore("S_ACT")
    S_DVE = nc.alloc_semaphore("S_DVE")
    S_POOL = nc.alloc_semaphore("S_POOL")
    S_OUT = nc.alloc_semaphore("S_OUT")

    cnt = {"pe": 0, "act": 0, "dve": 0, "pool": 0, "in": 0, "out": 0}

    # ---- input DMAs: weights first, then x chunk by chunk, so the
    # prologue only gates on what window 0 actually reads ----
    for cell in whh_d:
        nc.sync.dma_start(out=whh_s[cell].ap()[:, :],
                          in_=whh_d[cell].ap()[:, :]).then_inc(S_IN, 16)
        nc.sync.dma_start(out=bias_s[cell].ap()[:, :],
                          in_=bias_d[cell].ap()[:, :]).then_inc(S_IN, 16)
        cnt["in"] += 32
        for q, dd in enumerate(wih_d[cell]):
            nc.sync.dma_start(out=wih_s[cell][q].ap()[:, :],
                              in_=dd.ap()[:, :]).then_inc(S_IN, 16)
            cnt["in"] += 16
    ndma = 8
    chunk = TB // ndma
    in_after_chunk = []
    for i in range(ndma):
        for src_t, dst_t in ((xT_d, xT), (xTr_d, xTr)):
            nc.sync.dma_start(out=dst_t.ap()[:, i * chunk:(i + 1) * chunk],
                              in_=src_t.ap()[:, i * chunk:(i + 1) * chunk]
                              ).then_inc(S_IN, 16)
            cnt["in"] += 16
        in_after_chunk.append(cnt["in"])

    nc.vector.memset(ones_row.ap()[:, :], 1.0).then_inc(S_DVE, 1)
    nc.vector.memset(c_init.ap()[:, :], 0.0).then_inc(S_DVE, 1)
    cnt["dve"] += 2
    # gate Pool on c_init zero (m1 of step 0 reads it)
    nc.gpsimd.wait_ge(S_DVE, cnt["dve"])

    W = CH * BL  # columns per gate region in a bank / x columns per window

    def chunk_for_cols(last_col):
        return min(ndma - 1, last_col // chunk)

    # PE prologue gates: weights + x chunks for windows 0-1 + ones_row ready
    last_ck = chunk_for_cols(2 * W - 1)
    nc.tensor.wait_ge(S_IN, in_after_chunk[last_ck])
    nc.tensor.wait_ge(S_DVE, cnt["dve"])

    act_after_win = {}   # (lay, ci, window) -> S_ACT count after last sigma

    for lay in (1, 2):
        cells = (f"a{lay}", f"b{lay}")
        if lay == 1:
            rhs_src = {0: [(xT, 0)], 1: [(xTr, 0)]}
        else:
            rhs_src = {0: [(h_t[(1, 0)], 0), (h1bt, 0)],
                       1: [(h1ar, 0), (h_t[(1, 1)], 0)]}

        def precompute(ci, j, inc_pe=False):
            ps = pb[ci][j % 2].ap()
            war = act_after_win.pop((lay, ci, j), None)
            first = True
            for g in range(G):
                reg = g * W
                for q, (src, base) in enumerate(rhs_src[ci]):
                    mm = nc.tensor.matmul(
                        ps[:, reg:reg + W],
                        wih_s[cells[ci]][q].ap()[:, g * H:(g + 1) * H],
                        src.ap()[:, base + j * W:base + (j + 1) * W],
                        start=first, stop=False)
                    if first and war is not None:
                        mm._wait_ge(S_ACT, war)
                    first = False
            for g in range(G):
                mm = nc.tensor.matmul(
                    ps[:, g * W:g * W + W],
                    bias_s[cells[ci]].ap()[:, g * H:(g + 1) * H],
                    ones_row.ap()[:, :],
                    start=False, stop=(g == G - 1))
            if inc_pe:
                mm.then_inc(S_PE, 1)
                cnt["pe"] += 1
            return cnt["pe"]

        if lay == 2:
            # layer boundary: h1 / h1bt / h1ar fully written
            nc.tensor.wait_ge(S_DVE, cnt["dve"])
            nc.tensor.wait_ge(S_POOL, cnt["pool"])
        pe_cnt = {}
        for ci in (0, 1):
            precompute(ci, 0, inc_pe=True)
            pe_cnt[ci] = cnt["pe"]
            precompute(ci, 1)

        h_cnt = {0: None, 1: None}
        c_cnt = {}
        c_prev = {0: c_init.ap()[:, :], 1: c_init.ap()[:, :]}
        for k in range(t_len):
            j, slot = k // CH, k % CH
            # --- PE: recurrence matmuls, chain A then B ---
            if k > 0:
                for ci in (0, 1):
                    ps = pb[ci][j % 2].ap()
                    hprev = h_t[(lay, ci)].ap()[:, (k - 1) * BL:k * BL]
                    first = True
                    for g in range(G):
                        off = g * W + slot * BL
                        mm = nc.tensor.matmul(
                            ps[:, off:off + BL],
                            whh_s[cells[ci]].ap()[:, g * H:(g + 1) * H],
                            hprev, start=False, stop=False,
                            skip_group_check=True)
                        if first:
                            mm._wait_ge(S_DVE, h_cnt[ci])
                            first = False
                    mm.then_inc(S_PE, 1)
                    cnt["pe"] += 1
                    pe_cnt[ci] = cnt["pe"]
            # --- ACT: sigmas (A then B) ---
            sig_cnt = {}
            for ci in (0, 1):
                ps = pb[ci][j % 2].ap()
                s = s_r[ci][k % 2].ap()
                ps_view = ps[:, :].rearrange(
                    "p (g t b) -> p g t b", g=G, t=CH)[:, :, slot, :]
                s_view = s[:, :].rearrange("p (g b) -> p g b", g=G)
                sig = nc.scalar.activation(s_view, ps_view, AF.Sigmoid)
                sig._wait_ge(S_PE, pe_cnt[ci])
                sig.then_inc(S_ACT, 1)
                cnt["act"] += 1
                sig_cnt[ci] = cnt["act"]
                if slot == CH - 1:
                    act_after_win[(lay, ci, j + 2)] = cnt["act"]
            # --- DVE m2 + Pool m1, then DVE c (A then B) ---
            m1_cnt = {}
            for ci in (0, 1):
                s = s_r[ci][k % 2].ap()
                si, sf = s[:, 0:BL], s[:, BL:2 * BL]
                s2g = s[:, 2 * BL:3 * BL]
                m2 = m2_r[ci][k % 2].ap()
                op = nc.vector.scalar_tensor_tensor(
                    m2[:, :], s2g, 0.5, si, ALU.subtract, ALU.mult)
                op._wait_ge(S_ACT, sig_cnt[ci])
                m1 = m1_r[ci][k % 2].ap()
                m1op = nc.gpsimd.tensor_tensor(m1[:, :], sf, c_prev[ci],
                                               ALU.mult)
                m1op._wait_ge(S_ACT, sig_cnt[ci])
                m1op.then_inc(S_POOL, 1)
                cnt["pool"] += 1
                c = c_r[ci][k % 2].ap()
                cop = nc.vector.scalar_tensor_tensor(
                    c[:, :], m2[:, :], 2.0, m1[:, :], ALU.mult, ALU.add)
                cop._wait_ge(S_POOL, cnt["pool"])
                cop.then_inc(S_DVE, 1)
                cnt["dve"] += 1
                c_cnt[ci] = cnt["dve"]
                c_prev[ci] = c[:, :]
            # --- ACT tanh (A then B) ---
            th_cnt = {}
            for ci in (0, 1):
                th = th_r[ci][k % 2].ap()
                ti = nc.scalar.activation(th[:, :], c_r[ci][k % 2].ap()[:, :],
                                          AF.Tanh)
                ti._wait_ge(S_DVE, c_cnt[ci])
                ti.then_inc(S_ACT, 1)
                cnt["act"] += 1
                th_cnt[ci] = cnt["act"]
            # --- DVE h-mult (A then B) ---
            for ci in (0, 1):
                s = s_r[ci][k % 2].ap()
                so = s[:, 3 * BL:4 * BL]
                hop = nc.vector.tensor_tensor(
                    h_t[(lay, ci)].ap()[:, k * BL:(k + 1) * BL],
                    so, th_r[ci][k % 2].ap()[:, :], ALU.mult)
                hop._wait_ge(S_ACT, th_cnt[ci])
                hop.then_inc(S_DVE, 1)
                cnt["dve"] += 1
                h_cnt[ci] = cnt["dve"]
            # --- Pool copies for layer-2 precompute layouts ---
            if lay == 1:
                tau = t_len - 1 - k
                cp1 = nc.gpsimd.tensor_copy(
                    h1bt.ap()[:, tau * BL:(tau + 1) * BL],
                    h_t[(1, 1)].ap()[:, k * BL:(k + 1) * BL])
                cp1._wait_ge(S_DVE, h_cnt[1])
                cp1.then_inc(S_POOL, 1)
                cp2 = nc.gpsimd.tensor_copy(
                    h1ar.ap()[:, tau * BL:(tau + 1) * BL],
                    h_t[(1, 0)].ap()[:, k * BL:(k + 1) * BL])
                cp2._wait_ge(S_DVE, h_cnt[0])
                cp2.then_inc(S_POOL, 1)
                cnt["pool"] += 2
            # --- precompute next window ---
            if slot == CH // 2 - 1 and j + 1 < nw:
                if lay == 1:
                    ck = chunk_for_cols((j + 2) * W - 1)
                    if ck > last_ck:
                        # gate PE on the x chunks window j+1 reads
                        nc.tensor.wait_ge(S_IN, in_after_chunk[ck])
                        last_ck = ck
                for ci in (0, 1):
                    precompute(ci, j + 1)
            # --- stream output ---
            if lay == 2:
                ock = t_len // 8
                if (k + 1) % ock == 0:
                    lo = (k + 1 - ock) * BL
                    hi = (k + 1) * BL
                    for ci in (0, 1):
                        dma = nc.sync.dma_start(
                            out=o2_d.ap()[:, ci * TB + lo:ci * TB + hi],
                            in_=h_t[(2, ci)].ap()[:, lo:hi])
                        dma._wait_ge(S_DVE, h_cnt[ci])
                        dma.then_inc(S_OUT, 16)
                        cnt["out"] += 16

    nc.sync.wait_ge(S_OUT, cnt["out"])
    nc.all_engine_barrier()
    nc.compile()
    return nc


def _prep_weights(Wih, Whh, bih, bhh):
    """Host-side weight massaging: transpose, gate-scale (g-gate x2), bf16."""
    gscale = np.array([1.0, 1.0, 2.0, 1.0], np.float32)
    fourh, ind = Wih.shape
    wihT = np.ascontiguousarray(Wih.T).astype(np.float32)      # [in, 4H]
    whhT = np.ascontiguousarray(Whh.T).astype(np.float32)      # [H, 4H]
    bias = (bih + bhh).astype(np.float32)                      # [4H]
    for g in range(G):
        sl = slice(g * H, (g + 1) * H)
        wihT[:, sl] *= gscale[g]
        whhT[:, sl] *= gscale[g]
        bias[sl] *= gscale[g]
    nq = ind // H
    wih_chunks = [np.ascontiguousarray(wihT[q * H:(q + 1) * H]).astype(BF16_NP)
                  for q in range(nq)]
    bias_row = bias.reshape(1, G * H).astype(BF16_NP)           # [1, 4H]
    return wih_chunks, whhT.astype(BF16_NP), bias_row


def core_xT(xs, t_len):
    """Per-core input layouts: xs [BL, T, D] -> xT, xTr both [D, T*BL]."""
    fw = xs.transpose(2, 1, 0).reshape(D, t_len * BL)
    bw = xs[:, ::-1, :].transpose(2, 1, 0).reshape(D, t_len * BL)
    return (np.ascontiguousarray(fw).astype(BF16_NP),
            np.ascontiguousarray(bw).astype(BF16_NP))


def core_gather(res_c, t_len):
    """Per-core output: o2 [H, 2*T*BL] (bw half step-indexed) -> [BL,T,2H]."""
    o = np.asarray(res_c["o2"]).astype(np.float32)
    TB = t_len * BL
    out = np.empty((BL, t_len, 2 * H), np.float32)
    fw = o[:, :TB].reshape(H, t_len, BL)
    bw = o[:, TB:].reshape(H, t_len, BL)[:, ::-1, :]  # step k -> time T-1-k
    out[:, :, :H] = fw.transpose(2, 1, 0)
    out[:, :, H:] = bw.transpose(2, 1, 0)
    return out


_PROG_CACHE = {}


def prepare_in_maps(x, kw):
    x = np.asarray(x, np.float32)
    t_len = x.shape[1]
    cells = {"a1": (kw["Wih_fw1"], kw["Whh_fw1"], kw["bih_fw1"], kw["bhh_fw1"]),
             "b1": (kw["Wih_bw1"], kw["Whh_bw1"], kw["bih_bw1"], kw["bhh_bw1"]),
             "a2": (kw["Wih_fw2"], kw["Whh_fw2"], kw["bih_fw2"], kw["bhh_fw2"]),
             "b2": (kw["Wih_bw2"], kw["Whh_bw2"], kw["bih_bw2"], kw["bhh_bw2"])}
    wmaps = {}
    for cell, (Wih, Whh, bih, bhh) in cells.items():
        wih_chunks, whhT, bias_row = _prep_weights(
            np.asarray(Wih, np.float32), np.asarray(Whh, np.float32),
            np.asarray(bih, np.float32), np.asarray(bhh, np.float32))
        wmaps[f"whhT_{cell}"] = whhT
        wmaps[f"bias_{cell}"] = bias_row
        for q, wc in enumerate(wih_chunks):
            wmaps[f"wihT_{cell}_{q}"] = wc

    core_ids = list(range(NCORES))
    in_maps = []
    for c in core_ids:
        fw, bw = core_xT(x[c * BL:(c + 1) * BL], t_len)
        m = {"xT": fw, "xTr": bw}
        m.update(wmaps)
        in_maps.append(m)
    return in_maps, core_ids


def kernel(x, lengths, **kw):
    x = np.asarray(x, np.float32)
    t_len = x.shape[1]
    in_maps, core_ids = prepare_in_maps(x, kw)
    if t_len not in _PROG_CACHE:
        _PROG_CACHE[t_len] = build_program(t_len)
    nc = _PROG_CACHE[t_len]
    return _execute(nc, in_maps, core_ids, t_len)[0]


def _execute(nc, in_maps, core_ids, t_len, **run_kwargs):
    r = run_bass_kernel_spmd(nc, in_maps, core_ids, **run_kwargs)
    out = np.empty((B, t_len, 2 * H), np.float32)
    for c in core_ids:
        out[c * BL:(c + 1) * BL] = core_gather(r.results[c], t_len)
    return out, r


# revision 26
# speedup vs baseline: 1.1282x; 1.0001x over previous
"""Bass/Trainium2 kernel for a 2-layer bidirectional LSTM (CustomBiLSTM).

Strategy: data-parallel over batch across 8 NeuronCores (B=64 -> 8 per core).
Per core, each layer runs its forward (A) and backward (B) recurrent chains
as two INDEPENDENT instruction streams, phase-staggered by construction:
both loops have identical latency, so a constant offset is self-sustaining,
and the smaller per-chain ops ([128, 8..32]) make the serial chain shorter
than a lockstep-fused one.

Per chain and step:
    PE (4 recurrent matmuls) -> ACT sigmoid (4 gates) -> DVE m2 || Pool m1
      -> DVE c -> ACT tanh -> DVE h-mult -> PE ...

Synchronization is HAND-PLACED raw-bass semaphores (no Tile auto-sync):
each instruction carries at most ONE attached semaphore wait so engine
sequencers never block on standalone EventSemaphore instructions, keeping
decode off the critical path.  Same-engine ordering rides on queue order;
the c-op's Pool-semaphore wait also time-separates it from m2's in-flight
same-engine SBUF write.  Ring-buffer reuse (s/m2/m1/c/th, PSUM banks) is
safe transitively through each chain's dependency chain.

Gate pre-activations for a window of CH=16 steps live in one PSUM bank per
chain as [128, 4*CH*8]; input projections (Wih @ x) plus bias (rank-1
matmul against a ones row) are precomputed a half-window ahead, off the
critical path.

The backward chain's inputs/outputs are stored STEP-indexed (time
reversed).  The host supplies a time-reversed copy of x (xTr) so layer-1
precompute uses only positive strides; during layer 1, idle Pool-engine
copies maintain time-indexed/reversed views of h1 for layer-2 precompute.
The host un-reverses the backward half of the output.

The g-gate weights are pre-scaled by 2 on the host so a single Sigmoid
activation covers all 4 gates (tanh(z) = 2*sigmoid(2z) - 1); the affine
fix-up is fused into the DVE cell-state update.
"""

import numpy as np
import ml_dtypes

try:
    import concourse.bass as bass
except ImportError:
    import sys
    sys.path.insert(0, "/opt/trn_rl_repo")
    import concourse.bass as bass

import concourse.bacc as bacc
from concourse import mybir
from concourse.bass_utils import run_bass_kernel_spmd

F32 = mybir.dt.float32
BF16 = mybir.dt.bfloat16
AF = mybir.ActivationFunctionType
ALU = mybir.AluOpType
BF16_NP = ml_dtypes.bfloat16

H = 128          # hidden dim
D = 128          # input dim
B = 64           # global batch
T = 1024         # sequence length
NCORES = 8
BL = B // NCORES  # per-core batch = 8
G = 4            # gates (i, f, g, o)
CH = 512 // (G * BL)  # window steps per PSUM bank per chain (=16)


def build_program(t_len=T):
    nw = t_len // CH
    nc = bacc.Bacc("TRN2", target_bir_lowering=False, debug=False)
    TB = t_len * BL

    # ---- DRAM I/O ----
    xT_d = nc.dram_tensor("xT", [D, TB], BF16, kind="ExternalInput")
    xTr_d = nc.dram_tensor("xTr", [D, TB], BF16, kind="ExternalInput")
    whh_d, wih_d, bias_d = {}, {}, {}
    for lay in (1, 2):
        for dirn in ("a", "b"):
            cell = f"{dirn}{lay}"
            whh_d[cell] = nc.dram_tensor(f"whhT_{cell}", [H, G * H], BF16,
                                         kind="ExternalInput")
            bias_d[cell] = nc.dram_tensor(f"bias_{cell}", [1, G * H], BF16,
                                          kind="ExternalInput")
            nchunk = 1 if lay == 1 else 2
            wih_d[cell] = [
                nc.dram_tensor(f"wihT_{cell}_{q}", [H, G * H], BF16,
                               kind="ExternalInput")
                for q in range(nchunk)
            ]
    o2_d = nc.dram_tensor("o2", [H, 2 * TB], BF16, kind="ExternalOutput")

    # ---- SBUF ----
    xT = nc.alloc_sbuf_tensor("xT_s", [D, TB], BF16)
    xTr = nc.alloc_sbuf_tensor("xTr_s", [D, TB], BF16)
    whh_s, wih_s, bias_s = {}, {}, {}
    for cell in whh_d:
        whh_s[cell] = nc.alloc_sbuf_tensor(f"whh_{cell}", [H, G * H], BF16)
        bias_s[cell] = nc.alloc_sbuf_tensor(f"biasr_{cell}", [1, G * H], BF16)
        wih_s[cell] = [
            nc.alloc_sbuf_tensor(f"wih_{cell}_{q}", [H, G * H], BF16)
            for q in range(len(wih_d[cell]))
        ]
    ones_row = nc.alloc_sbuf_tensor("ones_row", [1, CH * BL], BF16)
    c_init = nc.alloc_sbuf_tensor("c_init", [H, BL], F32)
    # h storage per layer+chain, STEP-indexed (bw: step k -> time T-1-k)
    h_t = {(lay, ci): nc.alloc_sbuf_tensor(f"h{lay}{ci}", [H, TB], BF16)
           for lay in (1, 2) for ci in (0, 1)}
    h1bt = nc.alloc_sbuf_tensor("h1bt", [H, TB], BF16)  # h1 bw, time-indexed
    h1ar = nc.alloc_sbuf_tensor("h1ar", [H, TB], BF16)  # h1 fw, step-reversed
    # work rings (depth 2) per chain
    s_r = {ci: [nc.alloc_sbuf_tensor(f"s{ci}_{i}", [H, G * BL], F32)
                for i in (0, 1)] for ci in (0, 1)}
    m2_r = {ci: [nc.alloc_sbuf_tensor(f"m2{ci}_{i}", [H, BL], F32)
                 for i in (0, 1)] for ci in (0, 1)}
    m1_r = {ci: [nc.alloc_sbuf_tensor(f"m1{ci}_{i}", [H, BL], F32)
                 for i in (0, 1)] for ci in (0, 1)}
    c_r = {ci: [nc.alloc_sbuf_tensor(f"c{ci}_{i}", [H, BL], F32)
                for i in (0, 1)] for ci in (0, 1)}
    th_r = {ci: [nc.alloc_sbuf_tensor(f"th{ci}_{i}", [H, BL], F32)
                 for i in (0, 1)] for ci in (0, 1)}
    # ---- PSUM: 2 banks per chain (window ping-pong) ----
    pb = {ci: [nc.alloc_psum_tensor(f"pb{ci}_{p}", [H, G * CH * BL], F32)
               for p in (0, 1)] for ci in (0, 1)}

    # ---- semaphores ----
    S_IN = nc.alloc_semaphore("S_IN")
    S_PE = nc.alloc_semaphore("S_PE")
    S_ACT = nc.alloc_semaphore("S_ACT")
    S_DVE = nc.alloc_semaphore("S_DVE")
    S_POOL = nc.alloc_semaphore("S_POOL")
    S_OUT = nc.alloc_semaphore("S_OUT")

    cnt = {"pe": 0, "act": 0, "dve": 0, "pool": 0, "in": 0, "out": 0}

    # ---- input DMAs: weights first, then x chunk by chunk, so the
    # prologue only gates on what window 0 actually reads ----
    for cell in whh_d:
        nc.sync.dma_start(out=whh_s[cell].ap()[:, :],
                          in_=whh_d[cell].ap()[:, :]).then_inc(S_IN, 16)
        nc.sync.dma_start(out=bias_s[cell].ap()[:, :],
                          in_=bias_d[cell].ap()[:, :]).then_inc(S_IN, 16)
        cnt["in"] += 32
        for q, dd in enumerate(wih_d[cell]):
            nc.sync.dma_start(out=wih_s[cell][q].ap()[:, :],
                              in_=dd.ap()[:, :]).then_inc(S_IN, 16)
            cnt["in"] += 16
    ndma = 8
    chunk = TB // ndma
    in_after_chunk = []
    for i in range(ndma):
        for src_t, dst_t in ((xT_d, xT), (xTr_d, xTr)):
            nc.sync.dma_start(out=dst_t.ap()[:, i * chunk:(i + 1) * chunk],
                              in_=src_t.ap()[:, i * chunk:(i + 1) * chunk]
                              ).then_inc(S_IN, 16)
            cnt["in"] += 16
        in_after_chunk.append(cnt["in"])

    nc.vector.memset(ones_row.ap()[:, :], 1.0).then_inc(S_DVE, 1)
    nc.vector.memset(c_init.ap()[:, :], 0.0).then_inc(S_DVE, 1)
    cnt["dve"] += 2
    # gate Pool on c_init zero (m1 of step 0 reads it)
    nc.gpsimd.wait_ge(S_DVE, cnt["dve"])

    W = CH * BL  # columns per gate region in a bank / x columns per window

    def chunk_for_cols(last_col):
        return min(ndma - 1, last_col // chunk)

    # PE prologue gates: weights + x chunks for windows 0-1 + ones_row ready
    last_ck = chunk_for_cols(2 * W - 1)
    nc.tensor.wait_ge(S_IN, in_after_chunk[last_ck])
    nc.tensor.wait_ge(S_DVE, cnt["dve"])

    act_after_win = {}   # (lay, ci, window) -> S_ACT count after last sigma

    for lay in (1, 2):
        cells = (f"a{lay}", f"b{lay}")
        if lay == 1:
            rhs_src = {0: [(xT, 0)], 1: [(xTr, 0)]}
        else:
            rhs_src = {0: [(h_t[(1, 0)], 0), (h1bt, 0)],
                       1: [(h1ar, 0), (h_t[(1, 1)], 0)]}

        def precompute_mms(ci, j, inc_pe=False):
            """Emission closures for window j's precompute, so the matmuls
            can be dribbled a few per step (a single burst would head-of-line
            block the recurrence matmuls behind it on the FIFO PE queue)."""
            ps = pb[ci][j % 2].ap()
            war = act_after_win.pop((lay, ci, j), None)
            out = []
            for g in range(G):
                reg = g * W
                for q, (src, base) in enumerate(rhs_src[ci]):
                    def mk(g=g, q=q, src=src, base=base, reg=reg,
                           first=not out, war=war):
                        mm = nc.tensor.matmul(
                            ps[:, reg:reg + W],
                            wih_s[cells[ci]][q].ap()[:, g * H:(g + 1) * H],
                            src.ap()[:, base + j * W:base + (j + 1) * W],
                            start=first, stop=False)
                        if first and war is not None:
                            mm._wait_ge(S_ACT, war)
                    out.append(mk)
            for g in range(G):
                def mk(g=g, last=(g == G - 1)):
                    mm = nc.tensor.matmul(
                        ps[:, g * W:g * W + W],
                        bias_s[cells[ci]].ap()[:, g * H:(g + 1) * H],
                        ones_row.ap()[:, :],
                        start=False, stop=last)
                    if last and inc_pe:
                        mm.then_inc(S_PE, 1)
                        cnt["pe"] += 1
                out.append(mk)
            return out

        def precompute(ci, j, inc_pe=False):
            for mk in precompute_mms(ci, j, inc_pe):
                mk()
            return cnt["pe"]

        if lay == 2:
            # layer boundary: h1 / h1bt / h1ar fully written
            nc.tensor.wait_ge(S_DVE, cnt["dve"])
            nc.tensor.wait_ge(S_POOL, cnt["pool"])
        pe_cnt = {}
        for ci in (0, 1):
            precompute(ci, 0, inc_pe=True)
            pe_cnt[ci] = cnt["pe"]
            precompute(ci, 1)

        pending = []
        npc = 2 if lay == 1 else 3  # precompute matmuls emitted per step
        h_cnt = {0: None, 1: None}
        c_cnt = {}
        c_prev = {0: c_init.ap()[:, :], 1: c_init.ap()[:, :]}
        for k in range(t_len):
            j, slot = k // CH, k % CH
            # --- PE: recurrence matmuls, chain A then B ---
            if k > 0:
                for ci in (0, 1):
                    ps = pb[ci][j % 2].ap()
                    hprev = h_t[(lay, ci)].ap()[:, (k - 1) * BL:k * BL]
                    first = True
                    for g in range(G):
                        off = g * W + slot * BL
                        mm = nc.tensor.matmul(
                            ps[:, off:off + BL],
                            whh_s[cells[ci]].ap()[:, g * H:(g + 1) * H],
                            hprev, start=False, stop=False,
                            skip_group_check=True)
                        if first:
                            mm._wait_ge(S_DVE, h_cnt[ci])
                            first = False
                    mm.then_inc(S_PE, 1)
                    cnt["pe"] += 1
                    pe_cnt[ci] = cnt["pe"]
            # --- ACT: sigmas (A then B) ---
            sig_cnt = {}
            for ci in (0, 1):
                ps = pb[ci][j % 2].ap()
                s = s_r[ci][k % 2].ap()
                ps_view = ps[:, :].rearrange(
                    "p (g t b) -> p g t b", g=G, t=CH)[:, :, slot, :]
                s_view = s[:, :].rearrange("p (g b) -> p g b", g=G)
                sig = nc.scalar.activation(s_view, ps_view, AF.Sigmoid)
                sig._wait_ge(S_PE, pe_cnt[ci])
                sig.then_inc(S_ACT, 1)
                cnt["act"] += 1
                sig_cnt[ci] = cnt["act"]
                if slot == CH - 1:
                    act_after_win[(lay, ci, j + 2)] = cnt["act"]
            # --- DVE m2 + Pool m1, then DVE c (A then B) ---
            m1_cnt = {}
            for ci in (0, 1):
                s = s_r[ci][k % 2].ap()
                si, sf = s[:, 0:BL], s[:, BL:2 * BL]
                s2g = s[:, 2 * BL:3 * BL]
                m2 = m2_r[ci][k % 2].ap()
                op = nc.vector.scalar_tensor_tensor(
                    m2[:, :], s2g, 0.5, si, ALU.subtract, ALU.mult)
                op._wait_ge(S_ACT, sig_cnt[ci])
                m1 = m1_r[ci][k % 2].ap()
                m1op = nc.gpsimd.tensor_tensor(m1[:, :], sf, c_prev[ci],
                                               ALU.mult)
                m1op._wait_ge(S_ACT, sig_cnt[ci])
                m1op.then_inc(S_POOL, 1)
                cnt["pool"] += 1
                c = c_r[ci][k % 2].ap()
                cop = nc.vector.scalar_tensor_tensor(
                    c[:, :], m2[:, :], 2.0, m1[:, :], ALU.mult, ALU.add)
                cop._wait_ge(S_POOL, cnt["pool"])
                cop.then_inc(S_DVE, 1)
                cnt["dve"] += 1
                c_cnt[ci] = cnt["dve"]
                c_prev[ci] = c[:, :]
            # --- ACT tanh (A then B) ---
            th_cnt = {}
            for ci in (0, 1):
                th = th_r[ci][k % 2].ap()
                ti = nc.scalar.activation(th[:, :], c_r[ci][k % 2].ap()[:, :],
                                          AF.Tanh)
                ti._wait_ge(S_DVE, c_cnt[ci])
                ti.then_inc(S_ACT, 1)
                cnt["act"] += 1
                th_cnt[ci] = cnt["act"]
            # --- DVE h-mult (A then B) ---
            for ci in (0, 1):
                s = s_r[ci][k % 2].ap()
                so = s[:, 3 * BL:4 * BL]
                hop = nc.vector.tensor_tensor(
                    h_t[(lay, ci)].ap()[:, k * BL:(k + 1) * BL],
                    so, th_r[ci][k % 2].ap()[:, :], ALU.mult)
                hop._wait_ge(S_ACT, th_cnt[ci])
                hop.then_inc(S_DVE, 1)
                cnt["dve"] += 1
                h_cnt[ci] = cnt["dve"]
            # --- Pool copies for layer-2 precompute layouts ---
            if lay == 1:
                tau = t_len - 1 - k
                cp1 = nc.gpsimd.tensor_copy(
                    h1bt.ap()[:, tau * BL:(tau + 1) * BL],
                    h_t[(1, 1)].ap()[:, k * BL:(k + 1) * BL])
                cp1._wait_ge(S_DVE, h_cnt[1])
                cp1.then_inc(S_POOL, 1)
                cp2 = nc.gpsimd.tensor_copy(
                    h1ar.ap()[:, tau * BL:(tau + 1) * BL],
                    h_t[(1, 0)].ap()[:, k * BL:(k + 1) * BL])
                cp2._wait_ge(S_DVE, h_cnt[0])
                cp2.then_inc(S_POOL, 1)
                cnt["pool"] += 2
            # --- precompute next window: dribble a few matmuls per step ---
            if slot == 3 and j + 1 < nw:
                if lay == 1:
                    ck = chunk_for_cols((j + 2) * W - 1)
                    if ck > last_ck:
                        # gate PE on the x chunks window j+1 reads
                        nc.tensor.wait_ge(S_IN, in_after_chunk[ck])
                        last_ck = ck
                for ci in (0, 1):
                    pending.extend(precompute_mms(ci, j + 1))
            npop = min(len(pending), npc)
            for mk in pending[:npop]:
                mk()
            del pending[:npop]
            # --- stream output ---
            if lay == 2:
                ock = t_len // 8
                if (k + 1) % ock == 0:
                    lo = (k + 1 - ock) * BL
                    hi = (k + 1) * BL
                    for ci in (0, 1):
                        dma = nc.sync.dma_start(
                            out=o2_d.ap()[:, ci * TB + lo:ci * TB + hi],
                            in_=h_t[(2, ci)].ap()[:, lo:hi])
                        dma._wait_ge(S_DVE, h_cnt[ci])
                        dma.then_inc(S_OUT, 16)
                        cnt["out"] += 16

    nc.sync.wait_ge(S_OUT, cnt["out"])
    nc.all_engine_barrier()
    nc.compile()
    return nc


def _prep_weights(Wih, Whh, bih, bhh):
    """Host-side weight massaging: transpose, gate-scale (g-gate x2), bf16."""
    gscale = np.array([1.0, 1.0, 2.0, 1.0], np.float32)
    fourh, ind = Wih.shape
    wihT = np.ascontiguousarray(Wih.T).astype(np.float32)      # [in, 4H]
    whhT = np.ascontiguousarray(Whh.T).astype(np.float32)      # [H, 4H]
    bias = (bih + bhh).astype(np.float32)                      # [4H]
    for g in range(G):
        sl = slice(g * H, (g + 1) * H)
        wihT[:, sl] *= gscale[g]
        whhT[:, sl] *= gscale[g]
        bias[sl] *= gscale[g]
    nq = ind // H
    wih_chunks = [np.ascontiguousarray(wihT[q * H:(q + 1) * H]).astype(BF16_NP)
                  for q in range(nq)]
    bias_row = bias.reshape(1, G * H).astype(BF16_NP)           # [1, 4H]
    return wih_chunks, whhT.astype(BF16_NP), bias_row


def core_xT(xs, t_len):
    """Per-core input layouts: xs [BL, T, D] -> xT, xTr both [D, T*BL]."""
    fw = xs.transpose(2, 1, 0).reshape(D, t_len * BL)
    bw = xs[:, ::-1, :].transpose(2, 1, 0).reshape(D, t_len * BL)
    return (np.ascontiguousarray(fw).astype(BF16_NP),
            np.ascontiguousarray(bw).astype(BF16_NP))


def core_gather(res_c, t_len):
    """Per-core output: o2 [H, 2*T*BL] (bw half step-indexed) -> [BL,T,2H]."""
    o = np.asarray(res_c["o2"]).astype(np.float32)
    TB = t_len * BL
    out = np.empty((BL, t_len, 2 * H), np.float32)
    fw = o[:, :TB].reshape(H, t_len, BL)
    bw = o[:, TB:].reshape(H, t_len, BL)[:, ::-1, :]  # step k -> time T-1-k
    out[:, :, :H] = fw.transpose(2, 1, 0)
    out[:, :, H:] = bw.transpose(2, 1, 0)
    return out


_PROG_CACHE = {}


def prepare_in_maps(x, kw):
    x = np.asarray(x, np.float32)
    t_len = x.shape[1]
    cells = {"a1": (kw["Wih_fw1"], kw["Whh_fw1"], kw["bih_fw1"], kw["bhh_fw1"]),
             "b1": (kw["Wih_bw1"], kw["Whh_bw1"], kw["bih_bw1"], kw["bhh_bw1"]),
             "a2": (kw["Wih_fw2"], kw["Whh_fw2"], kw["bih_fw2"], kw["bhh_fw2"]),
             "b2": (kw["Wih_bw2"], kw["Whh_bw2"], kw["bih_bw2"], kw["bhh_bw2"])}
    wmaps = {}
    for cell, (Wih, Whh, bih, bhh) in cells.items():
        wih_chunks, whhT, bias_row = _prep_weights(
            np.asarray(Wih, np.float32), np.asarray(Whh, np.float32),
            np.asarray(bih, np.float32), np.asarray(bhh, np.float32))
        wmaps[f"whhT_{cell}"] = whhT
        wmaps[f"bias_{cell}"] = bias_row
        for q, wc in enumerate(wih_chunks):
            wmaps[f"wihT_{cell}_{q}"] = wc

    core_ids = list(range(NCORES))
    in_maps = []
    for c in core_ids:
        fw, bw = core_xT(x[c * BL:(c + 1) * BL], t_len)
        m = {"xT": fw, "xTr": bw}
        m.update(wmaps)
        in_maps.append(m)
    return in_maps, core_ids


def kernel(x, lengths, **kw):
    x = np.asarray(x, np.float32)
    t_len = x.shape[1]
    in_maps, core_ids = prepare_in_maps(x, kw)
    if t_len not in _PROG_CACHE:
        _PROG_CACHE[t_len] = build_program(t_len)
    nc = _PROG_CACHE[t_len]
    return _execute(nc, in_maps, core_ids, t_len)[0]


def _execute(nc, in_maps, core_ids, t_len, **run_kwargs):
    r = run_bass_kernel_spmd(nc, in_maps, core_ids, **run_kwargs)
    out = np.empty((B, t_len, 2 * H), np.float32)
    for c in core_ids:
        out[c * BL:(c + 1) * BL] = core_gather(r.results[c], t_len)
    return out, r


# revision 27
# speedup vs baseline: 1.1291x; 1.0008x over previous
"""Bass/Trainium2 kernel for a 2-layer bidirectional LSTM (CustomBiLSTM).

Strategy: data-parallel over batch across 8 NeuronCores (B=64 -> 8 per core).
Per core, each layer runs its forward (A) and backward (B) recurrent chains
as two INDEPENDENT instruction streams, phase-staggered by construction:
both loops have identical latency, so a constant offset is self-sustaining,
and the smaller per-chain ops ([128, 8..32]) make the serial chain shorter
than a lockstep-fused one.

Per chain and step:
    PE (4 recurrent matmuls) -> ACT sigmoid (4 gates) -> DVE m2 || Pool m1
      -> DVE c -> ACT tanh -> DVE h-mult -> PE ...

Synchronization is HAND-PLACED raw-bass semaphores (no Tile auto-sync):
each instruction carries at most ONE attached semaphore wait so engine
sequencers never block on standalone EventSemaphore instructions, keeping
decode off the critical path.  Same-engine ordering rides on queue order;
the c-op's Pool-semaphore wait also time-separates it from m2's in-flight
same-engine SBUF write.  Ring-buffer reuse (s/m2/m1/c/th, PSUM banks) is
safe transitively through each chain's dependency chain.

Gate pre-activations for a window of CH=16 steps live in one PSUM bank per
chain as [128, 4*CH*8]; input projections (Wih @ x) plus bias (rank-1
matmul against a ones row) are precomputed a half-window ahead, off the
critical path.

The backward chain's inputs/outputs are stored STEP-indexed (time
reversed).  The host supplies a time-reversed copy of x (xTr) so layer-1
precompute uses only positive strides; during layer 1, idle Pool-engine
copies maintain time-indexed/reversed views of h1 for layer-2 precompute.
The host un-reverses the backward half of the output.

The g-gate weights are pre-scaled by 2 on the host so a single Sigmoid
activation covers all 4 gates (tanh(z) = 2*sigmoid(2z) - 1); the affine
fix-up is fused into the DVE cell-state update.
"""

import numpy as np
import ml_dtypes

try:
    import concourse.bass as bass
except ImportError:
    import sys
    sys.path.insert(0, "/opt/trn_rl_repo")
    import concourse.bass as bass

import concourse.bacc as bacc
from concourse import mybir
from concourse.bass_utils import run_bass_kernel_spmd

F32 = mybir.dt.float32
BF16 = mybir.dt.bfloat16
AF = mybir.ActivationFunctionType
ALU = mybir.AluOpType
BF16_NP = ml_dtypes.bfloat16

H = 128          # hidden dim
D = 128          # input dim
B = 64           # global batch
T = 1024         # sequence length
NCORES = 8
BL = B // NCORES  # per-core batch = 8
G = 4            # gates (i, f, g, o)
CH = 512 // (G * BL)  # window steps per PSUM bank per chain (=16)


def build_program(t_len=T):
    nw = t_len // CH
    nc = bacc.Bacc("TRN2", target_bir_lowering=False, debug=False)
    TB = t_len * BL

    # ---- DRAM I/O ----
    xT_d = nc.dram_tensor("xT", [D, TB], BF16, kind="ExternalInput")
    xTr_d = nc.dram_tensor("xTr", [D, TB], BF16, kind="ExternalInput")
    whh_d, wih_d, bias_d = {}, {}, {}
    for lay in (1, 2):
        for dirn in ("a", "b"):
            cell = f"{dirn}{lay}"
            whh_d[cell] = nc.dram_tensor(f"whhT_{cell}", [H, G * H], BF16,
                                         kind="ExternalInput")
            bias_d[cell] = nc.dram_tensor(f"bias_{cell}", [1, G * H], BF16,
                                          kind="ExternalInput")
            nchunk = 1 if lay == 1 else 2
            wih_d[cell] = [
                nc.dram_tensor(f"wihT_{cell}_{q}", [H, G * H], BF16,
                               kind="ExternalInput")
                for q in range(nchunk)
            ]
    o2_d = nc.dram_tensor("o2", [H, 2 * TB], BF16, kind="ExternalOutput")

    # ---- SBUF ----
    xT = nc.alloc_sbuf_tensor("xT_s", [D, TB], BF16)
    xTr = nc.alloc_sbuf_tensor("xTr_s", [D, TB], BF16)
    whh_s, wih_s, bias_s = {}, {}, {}
    for cell in whh_d:
        whh_s[cell] = nc.alloc_sbuf_tensor(f"whh_{cell}", [H, G * H], BF16)
        bias_s[cell] = nc.alloc_sbuf_tensor(f"biasr_{cell}", [1, G * H], BF16)
        wih_s[cell] = [
            nc.alloc_sbuf_tensor(f"wih_{cell}_{q}", [H, G * H], BF16)
            for q in range(len(wih_d[cell]))
        ]
    ones_row = nc.alloc_sbuf_tensor("ones_row", [1, CH * BL], BF16)
    c_init = nc.alloc_sbuf_tensor("c_init", [H, BL], F32)
    # h storage per layer+chain, STEP-indexed (bw: step k -> time T-1-k)
    h_t = {(lay, ci): nc.alloc_sbuf_tensor(f"h{lay}{ci}", [H, TB], BF16)
           for lay in (1, 2) for ci in (0, 1)}
    h1bt = nc.alloc_sbuf_tensor("h1bt", [H, TB], BF16)  # h1 bw, time-indexed
    h1ar = nc.alloc_sbuf_tensor("h1ar", [H, TB], BF16)  # h1 fw, step-reversed
    # work rings (depth 2) per chain
    s_r = {ci: [nc.alloc_sbuf_tensor(f"s{ci}_{i}", [H, G * BL], F32)
                for i in (0, 1)] for ci in (0, 1)}
    m2_r = {ci: [nc.alloc_sbuf_tensor(f"m2{ci}_{i}", [H, BL], F32)
                 for i in (0, 1)] for ci in (0, 1)}
    m1_r = {ci: [nc.alloc_sbuf_tensor(f"m1{ci}_{i}", [H, BL], F32)
                 for i in (0, 1)] for ci in (0, 1)}
    c_r = {ci: [nc.alloc_sbuf_tensor(f"c{ci}_{i}", [H, BL], F32)
                for i in (0, 1)] for ci in (0, 1)}
    th_r = {ci: [nc.alloc_sbuf_tensor(f"th{ci}_{i}", [H, BL], F32)
                 for i in (0, 1)] for ci in (0, 1)}
    # ---- PSUM: 2 banks per chain (window ping-pong) ----
    pb = {ci: [nc.alloc_psum_tensor(f"pb{ci}_{p}", [H, G * CH * BL], F32)
               for p in (0, 1)] for ci in (0, 1)}

    # ---- semaphores ----
    S_IN = nc.alloc_semaphore("S_IN")
    S_PE = nc.alloc_semaphore("S_PE")
    S_ACT = nc.alloc_semaphore("S_ACT")
    S_DVE = nc.alloc_semaphore("S_DVE")
    S_POOL = nc.alloc_semaphore("S_POOL")
    S_OUT = nc.alloc_semaphore("S_OUT")

    cnt = {"pe": 0, "act": 0, "dve": 0, "pool": 0, "in": 0, "out": 0}

    # ---- input DMAs: weights first, then x chunk by chunk, so the
    # prologue only gates on what window 0 actually reads ----
    for cell in whh_d:
        nc.sync.dma_start(out=whh_s[cell].ap()[:, :],
                          in_=whh_d[cell].ap()[:, :]).then_inc(S_IN, 16)
        nc.sync.dma_start(out=bias_s[cell].ap()[:, :],
                          in_=bias_d[cell].ap()[:, :]).then_inc(S_IN, 16)
        cnt["in"] += 32
        for q, dd in enumerate(wih_d[cell]):
            nc.sync.dma_start(out=wih_s[cell][q].ap()[:, :],
                              in_=dd.ap()[:, :]).then_inc(S_IN, 16)
            cnt["in"] += 16
    ndma = 8
    chunk = TB // ndma
    in_after_chunk = []
    for i in range(ndma):
        for src_t, dst_t in ((xT_d, xT), (xTr_d, xTr)):
            nc.sync.dma_start(out=dst_t.ap()[:, i * chunk:(i + 1) * chunk],
                              in_=src_t.ap()[:, i * chunk:(i + 1) * chunk]
                              ).then_inc(S_IN, 16)
            cnt["in"] += 16
        in_after_chunk.append(cnt["in"])

    nc.vector.memset(ones_row.ap()[:, :], 1.0).then_inc(S_DVE, 1)
    nc.vector.memset(c_init.ap()[:, :], 0.0).then_inc(S_DVE, 1)
    cnt["dve"] += 2
    # gate Pool on c_init zero (m1 of step 0 reads it)
    nc.gpsimd.wait_ge(S_DVE, cnt["dve"])

    W = CH * BL  # columns per gate region in a bank / x columns per window

    def chunk_for_cols(last_col):
        return min(ndma - 1, last_col // chunk)

    # PE prologue gates: weights + x chunks for windows 0-1 + ones_row ready
    last_ck = chunk_for_cols(2 * W - 1)
    nc.tensor.wait_ge(S_IN, in_after_chunk[last_ck])
    nc.tensor.wait_ge(S_DVE, cnt["dve"])

    act_after_win = {}   # (lay, ci, window) -> S_ACT count after last sigma

    for lay in (1, 2):
        cells = (f"a{lay}", f"b{lay}")
        if lay == 1:
            rhs_src = {0: [(xT, 0)], 1: [(xTr, 0)]}
        else:
            rhs_src = {0: [(h_t[(1, 0)], 0), (h1bt, 0)],
                       1: [(h1ar, 0), (h_t[(1, 1)], 0)]}

        def precompute_mms(ci, j, inc_pe=False):
            """Emission closures for window j's precompute, so the matmuls
            can be dribbled a few per step (a single burst would head-of-line
            block the recurrence matmuls behind it on the FIFO PE queue)."""
            ps = pb[ci][j % 2].ap()
            war = act_after_win.pop((lay, ci, j), None)
            out = []
            for g in range(G):
                reg = g * W
                for q, (src, base) in enumerate(rhs_src[ci]):
                    def mk(g=g, q=q, src=src, base=base, reg=reg,
                           first=not out, war=war):
                        mm = nc.tensor.matmul(
                            ps[:, reg:reg + W],
                            wih_s[cells[ci]][q].ap()[:, g * H:(g + 1) * H],
                            src.ap()[:, base + j * W:base + (j + 1) * W],
                            start=first, stop=False)
                        if first and war is not None:
                            mm._wait_ge(S_ACT, war)
                    out.append(mk)
            for g in range(G):
                def mk(g=g, last=(g == G - 1)):
                    mm = nc.tensor.matmul(
                        ps[:, g * W:g * W + W],
                        bias_s[cells[ci]].ap()[:, g * H:(g + 1) * H],
                        ones_row.ap()[:, :],
                        start=False, stop=last)
                    if last and inc_pe:
                        mm.then_inc(S_PE, 1)
                        cnt["pe"] += 1
                out.append(mk)
            return out

        def precompute(ci, j, inc_pe=False):
            for mk in precompute_mms(ci, j, inc_pe):
                mk()
            return cnt["pe"]

        if lay == 2:
            # layer boundary: h1 / h1bt / h1ar fully written
            nc.tensor.wait_ge(S_DVE, cnt["dve"])
            nc.tensor.wait_ge(S_POOL, cnt["pool"])
        pe_cnt = {}
        pending = []
        for ci in (0, 1):
            precompute(ci, 0, inc_pe=True)
            pe_cnt[ci] = cnt["pe"]
            # window 1 is dribbled into the first steps instead of primed
            # as one blob (it is only read from step CH on)
            pending.extend(precompute_mms(ci, 1))
        npc = 2 if lay == 1 else 3  # precompute matmuls emitted per step
        h_cnt = {0: None, 1: None}
        c_cnt = {}
        c_prev = {0: c_init.ap()[:, :], 1: c_init.ap()[:, :]}
        for k in range(t_len):
            j, slot = k // CH, k % CH
            # --- PE: recurrence matmuls, chain A then B ---
            if k > 0:
                for ci in (0, 1):
                    ps = pb[ci][j % 2].ap()
                    hprev = h_t[(lay, ci)].ap()[:, (k - 1) * BL:k * BL]
                    first = True
                    for g in range(G):
                        off = g * W + slot * BL
                        mm = nc.tensor.matmul(
                            ps[:, off:off + BL],
                            whh_s[cells[ci]].ap()[:, g * H:(g + 1) * H],
                            hprev, start=False, stop=False,
                            skip_group_check=True)
                        if first:
                            mm._wait_ge(S_DVE, h_cnt[ci])
                            first = False
                    mm.then_inc(S_PE, 1)
                    cnt["pe"] += 1
                    pe_cnt[ci] = cnt["pe"]
            # --- ACT: sigmas (A then B) ---
            sig_cnt = {}
            for ci in (0, 1):
                ps = pb[ci][j % 2].ap()
                s = s_r[ci][k % 2].ap()
                ps_view = ps[:, :].rearrange(
                    "p (g t b) -> p g t b", g=G, t=CH)[:, :, slot, :]
                s_view = s[:, :].rearrange("p (g b) -> p g b", g=G)
                sig = nc.scalar.activation(s_view, ps_view, AF.Sigmoid)
                sig._wait_ge(S_PE, pe_cnt[ci])
                sig.then_inc(S_ACT, 1)
                cnt["act"] += 1
                sig_cnt[ci] = cnt["act"]
                if slot == CH - 1:
                    act_after_win[(lay, ci, j + 2)] = cnt["act"]
            # --- DVE m2 + Pool m1, then DVE c (A then B) ---
            m1_cnt = {}
            for ci in (0, 1):
                s = s_r[ci][k % 2].ap()
                si, sf = s[:, 0:BL], s[:, BL:2 * BL]
                s2g = s[:, 2 * BL:3 * BL]
                m2 = m2_r[ci][k % 2].ap()
                op = nc.vector.scalar_tensor_tensor(
                    m2[:, :], s2g, 0.5, si, ALU.subtract, ALU.mult)
                op._wait_ge(S_ACT, sig_cnt[ci])
                m1 = m1_r[ci][k % 2].ap()
                m1op = nc.gpsimd.tensor_tensor(m1[:, :], sf, c_prev[ci],
                                               ALU.mult)
                m1op._wait_ge(S_ACT, sig_cnt[ci])
                m1op.then_inc(S_POOL, 1)
                cnt["pool"] += 1
                c = c_r[ci][k % 2].ap()
                cop = nc.vector.scalar_tensor_tensor(
                    c[:, :], m2[:, :], 2.0, m1[:, :], ALU.mult, ALU.add)
                cop._wait_ge(S_POOL, cnt["pool"])
                cop.then_inc(S_DVE, 1)
                cnt["dve"] += 1
                c_cnt[ci] = cnt["dve"]
                c_prev[ci] = c[:, :]
            # --- ACT tanh (A then B) ---
            th_cnt = {}
            for ci in (0, 1):
                th = th_r[ci][k % 2].ap()
                ti = nc.scalar.activation(th[:, :], c_r[ci][k % 2].ap()[:, :],
                                          AF.Tanh)
                ti._wait_ge(S_DVE, c_cnt[ci])
                ti.then_inc(S_ACT, 1)
                cnt["act"] += 1
                th_cnt[ci] = cnt["act"]
            # --- DVE h-mult (A then B) ---
            for ci in (0, 1):
                s = s_r[ci][k % 2].ap()
                so = s[:, 3 * BL:4 * BL]
                hop = nc.vector.tensor_tensor(
                    h_t[(lay, ci)].ap()[:, k * BL:(k + 1) * BL],
                    so, th_r[ci][k % 2].ap()[:, :], ALU.mult)
                hop._wait_ge(S_ACT, th_cnt[ci])
                hop.then_inc(S_DVE, 1)
                cnt["dve"] += 1
                h_cnt[ci] = cnt["dve"]
            # --- Pool copies for layer-2 precompute layouts ---
            if lay == 1:
                tau = t_len - 1 - k
                cp1 = nc.gpsimd.tensor_copy(
                    h1bt.ap()[:, tau * BL:(tau + 1) * BL],
                    h_t[(1, 1)].ap()[:, k * BL:(k + 1) * BL])
                cp1._wait_ge(S_DVE, h_cnt[1])
                cp1.then_inc(S_POOL, 1)
                cp2 = nc.gpsimd.tensor_copy(
                    h1ar.ap()[:, tau * BL:(tau + 1) * BL],
                    h_t[(1, 0)].ap()[:, k * BL:(k + 1) * BL])
                cp2._wait_ge(S_DVE, h_cnt[0])
                cp2.then_inc(S_POOL, 1)
                cnt["pool"] += 2
            # --- precompute next window: dribble a few matmuls per step ---
            if slot == 3 and 2 <= j + 1 < nw:
                if lay == 1:
                    ck = chunk_for_cols((j + 2) * W - 1)
                    if ck > last_ck:
                        # gate PE on the x chunks window j+1 reads
                        nc.tensor.wait_ge(S_IN, in_after_chunk[ck])
                        last_ck = ck
                for ci in (0, 1):
                    pending.extend(precompute_mms(ci, j + 1))
            npop = min(len(pending), npc)
            for mk in pending[:npop]:
                mk()
            del pending[:npop]
            # --- stream output ---
            if lay == 2:
                ock = t_len // 8
                if (k + 1) % ock == 0:
                    lo = (k + 1 - ock) * BL
                    hi = (k + 1) * BL
                    for ci in (0, 1):
                        dma = nc.sync.dma_start(
                            out=o2_d.ap()[:, ci * TB + lo:ci * TB + hi],
                            in_=h_t[(2, ci)].ap()[:, lo:hi])
                        dma._wait_ge(S_DVE, h_cnt[ci])
                        dma.then_inc(S_OUT, 16)
                        cnt["out"] += 16

    nc.sync.wait_ge(S_OUT, cnt["out"])
    nc.all_engine_barrier()
    nc.compile()
    return nc


def _prep_weights(Wih, Whh, bih, bhh):
    """Host-side weight massaging: transpose, gate-scale (g-gate x2), bf16."""
    gscale = np.array([1.0, 1.0, 2.0, 1.0], np.float32)
    fourh, ind = Wih.shape
    wihT = np.ascontiguousarray(Wih.T).astype(np.float32)      # [in, 4H]
    whhT = np.ascontiguousarray(Whh.T).astype(np.float32)      # [H, 4H]
    bias = (bih + bhh).astype(np.float32)                      # [4H]
    for g in range(G):
        sl = slice(g * H, (g + 1) * H)
        wihT[:, sl] *= gscale[g]
        whhT[:, sl] *= gscale[g]
        bias[sl] *= gscale[g]
    nq = ind // H
    wih_chunks = [np.ascontiguousarray(wihT[q * H:(q + 1) * H]).astype(BF16_NP)
                  for q in range(nq)]
    bias_row = bias.reshape(1, G * H).astype(BF16_NP)           # [1, 4H]
    return wih_chunks, whhT.astype(BF16_NP), bias_row


def core_xT(xs, t_len):
    """Per-core input layouts: xs [BL, T, D] -> xT, xTr both [D, T*BL]."""
    fw = xs.transpose(2, 1, 0).reshape(D, t_len * BL)
    bw = xs[:, ::-1, :].transpose(2, 1, 0).reshape(D, t_len * BL)
    return (np.ascontiguousarray(fw).astype(BF16_NP),
            np.ascontiguousarray(bw).astype(BF16_NP))


def core_gather(res_c, t_len):
    """Per-core output: o2 [H, 2*T*BL] (bw half step-indexed) -> [BL,T,2H]."""
    o = np.asarray(res_c["o2"]).astype(np.float32)
    TB = t_len * BL
    out = np.empty((BL, t_len, 2 * H), np.float32)
    fw = o[:, :TB].reshape(H, t_len, BL)
    bw = o[:, TB:].reshape(H, t_len, BL)[:, ::-1, :]  # step k -> time T-1-k
    out[:, :, :H] = fw.transpose(2, 1, 0)
    out[:, :, H:] = bw.transpose(2, 1, 0)
    return out


_PROG_CACHE = {}


def prepare_in_maps(x, kw):
    x = np.asarray(x, np.float32)
    t_len = x.shape[1]
    cells = {"a1": (kw["Wih_fw1"], kw["Whh_fw1"], kw["bih_fw1"], kw["bhh_fw1"]),
             "b1": (kw["Wih_bw1"], kw["Whh_bw1"], kw["bih_bw1"], kw["bhh_bw1"]),
             "a2": (kw["Wih_fw2"], kw["Whh_fw2"], kw["bih_fw2"], kw["bhh_fw2"]),
             "b2": (kw["Wih_bw2"], kw["Whh_bw2"], kw["bih_bw2"], kw["bhh_bw2"])}
    wmaps = {}
    for cell, (Wih, Whh, bih, bhh) in cells.items():
        wih_chunks, whhT, bias_row = _prep_weights(
            np.asarray(Wih, np.float32), np.asarray(Whh, np.float32),
            np.asarray(bih, np.float32), np.asarray(bhh, np.float32))
        wmaps[f"whhT_{cell}"] = whhT
        wmaps[f"bias_{cell}"] = bias_row
        for q, wc in enumerate(wih_chunks):
            wmaps[f"wihT_{cell}_{q}"] = wc

    core_ids = list(range(NCORES))
    in_maps = []
    for c in core_ids:
        fw, bw = core_xT(x[c * BL:(c + 1) * BL], t_len)
        m = {"xT": fw, "xTr": bw}
        m.update(wmaps)
        in_maps.append(m)
    return in_maps, core_ids


def kernel(x, lengths, **kw):
    x = np.asarray(x, np.float32)
    t_len = x.shape[1]
    in_maps, core_ids = prepare_in_maps(x, kw)
    if t_len not in _PROG_CACHE:
        _PROG_CACHE[t_len] = build_program(t_len)
    nc = _PROG_CACHE[t_len]
    return _execute(nc, in_maps, core_ids, t_len)[0]


def _execute(nc, in_maps, core_ids, t_len, **run_kwargs):
    r = run_bass_kernel_spmd(nc, in_maps, core_ids, **run_kwargs)
    out = np.empty((B, t_len, 2 * H), np.float32)
    for c in core_ids:
        out[c * BL:(c + 1) * BL] = core_gather(r.results[c], t_len)
    return out, r


# revision 31
# speedup vs baseline: 1.1309x; 1.0016x over previous
"""Bass/Trainium2 kernel for a 2-layer bidirectional LSTM (CustomBiLSTM).

Strategy: data-parallel over batch across 8 NeuronCores (B=64 -> 8 per core).
Per core, each layer runs its forward (A) and backward (B) recurrent chains
as two INDEPENDENT instruction streams, phase-staggered by construction:
both loops have identical latency, so a constant offset is self-sustaining,
and the smaller per-chain ops ([128, 8..32]) make the serial chain shorter
than a lockstep-fused one.

Per chain and step:
    PE (4 recurrent matmuls) -> ACT sigmoid (4 gates) -> DVE m2 || Pool m1
      -> DVE c -> ACT tanh -> DVE h-mult -> PE ...

Synchronization is HAND-PLACED raw-bass semaphores (no Tile auto-sync):
each instruction carries at most ONE attached semaphore wait so engine
sequencers never block on standalone EventSemaphore instructions, keeping
decode off the critical path.  Same-engine ordering rides on queue order;
the c-op's Pool-semaphore wait also time-separates it from m2's in-flight
same-engine SBUF write.  Ring-buffer reuse (s/m2/m1/c/th, PSUM banks) is
safe transitively through each chain's dependency chain.

Gate pre-activations for a window of CH=16 steps live in one PSUM bank per
chain as [128, 4*CH*8]; input projections (Wih @ x) plus bias (rank-1
matmul against a ones row) are precomputed a half-window ahead, off the
critical path.

The backward chain's inputs/outputs are stored STEP-indexed (time
reversed).  The host supplies a time-reversed copy of x (xTr) so layer-1
precompute uses only positive strides; during layer 1, idle Pool-engine
copies maintain time-indexed/reversed views of h1 for layer-2 precompute.
The host un-reverses the backward half of the output.

The g-gate weights are pre-scaled by 2 on the host so a single Sigmoid
activation covers all 4 gates (tanh(z) = 2*sigmoid(2z) - 1); the affine
fix-up is fused into the DVE cell-state update.
"""

import numpy as np
import ml_dtypes

try:
    import concourse.bass as bass
except ImportError:
    import sys
    sys.path.insert(0, "/opt/trn_rl_repo")
    import concourse.bass as bass

import concourse.bacc as bacc
from concourse import mybir
from concourse.bass_utils import run_bass_kernel_spmd

F32 = mybir.dt.float32
BF16 = mybir.dt.bfloat16
AF = mybir.ActivationFunctionType
ALU = mybir.AluOpType
BF16_NP = ml_dtypes.bfloat16

H = 128          # hidden dim
D = 128          # input dim
B = 64           # global batch
T = 1024         # sequence length
NCORES = 8
BL = B // NCORES  # per-core batch = 8
G = 4            # gates (i, f, g, o)
CH = 512 // (G * BL)  # window steps per PSUM bank per chain (=16)


def build_program(t_len=T):
    nw = t_len // CH
    nc = bacc.Bacc("TRN2", target_bir_lowering=False, debug=False)
    TB = t_len * BL

    # ---- DRAM I/O ----
    xT_d = nc.dram_tensor("xT", [D, TB], BF16, kind="ExternalInput")
    xTr_d = nc.dram_tensor("xTr", [D, TB], BF16, kind="ExternalInput")
    whh_d, wih_d, bias_d = {}, {}, {}
    for lay in (1, 2):
        for dirn in ("a", "b"):
            cell = f"{dirn}{lay}"
            whh_d[cell] = nc.dram_tensor(f"whhT_{cell}", [H, G * H], BF16,
                                         kind="ExternalInput")
            bias_d[cell] = nc.dram_tensor(f"bias_{cell}", [1, G * H], BF16,
                                          kind="ExternalInput")
            nchunk = 1 if lay == 1 else 2
            wih_d[cell] = [
                nc.dram_tensor(f"wihT_{cell}_{q}", [H, G * H], BF16,
                               kind="ExternalInput")
                for q in range(nchunk)
            ]
    o2_d = nc.dram_tensor("o2", [H, 2 * TB], BF16, kind="ExternalOutput")

    # ---- SBUF ----
    xT = nc.alloc_sbuf_tensor("xT_s", [D, TB], BF16)
    xTr = nc.alloc_sbuf_tensor("xTr_s", [D, TB], BF16)
    whh_s, wih_s, bias_s = {}, {}, {}
    for cell in whh_d:
        whh_s[cell] = nc.alloc_sbuf_tensor(f"whh_{cell}", [H, G * H], BF16)
        bias_s[cell] = nc.alloc_sbuf_tensor(f"biasr_{cell}", [1, G * H], BF16)
        wih_s[cell] = [
            nc.alloc_sbuf_tensor(f"wih_{cell}_{q}", [H, G * H], BF16)
            for q in range(len(wih_d[cell]))
        ]
    ones_row = nc.alloc_sbuf_tensor("ones_row", [1, CH * BL], BF16)
    c_init = nc.alloc_sbuf_tensor("c_init", [H, BL], F32)
    # h storage per layer+chain, STEP-indexed (bw: step k -> time T-1-k)
    h_t = {(lay, ci): nc.alloc_sbuf_tensor(f"h{lay}{ci}", [H, TB], BF16)
           for lay in (1, 2) for ci in (0, 1)}
    h1bt = nc.alloc_sbuf_tensor("h1bt", [H, TB], BF16)  # h1 bw, time-indexed
    h1ar = nc.alloc_sbuf_tensor("h1ar", [H, TB], BF16)  # h1 fw, step-reversed
    # work rings (depth 2) per chain
    s_r = {ci: [nc.alloc_sbuf_tensor(f"s{ci}_{i}", [H, G * BL], F32)
                for i in (0, 1)] for ci in (0, 1)}
    m2_r = {ci: [nc.alloc_sbuf_tensor(f"m2{ci}_{i}", [H, BL], F32)
                 for i in (0, 1)] for ci in (0, 1)}
    m1_r = {ci: [nc.alloc_sbuf_tensor(f"m1{ci}_{i}", [H, BL], F32)
                 for i in (0, 1)] for ci in (0, 1)}
    c_r = {ci: [nc.alloc_sbuf_tensor(f"c{ci}_{i}", [H, BL], F32)
                for i in (0, 1)] for ci in (0, 1)}
    th_r = {ci: [nc.alloc_sbuf_tensor(f"th{ci}_{i}", [H, BL], F32)
                 for i in (0, 1)] for ci in (0, 1)}
    # ---- PSUM: 2 banks per chain (window ping-pong) ----
    pb = {ci: [nc.alloc_psum_tensor(f"pb{ci}_{p}", [H, G * CH * BL], F32)
               for p in (0, 1)] for ci in (0, 1)}

    # ---- semaphores ----
    S_IN = nc.alloc_semaphore("S_IN")
    S_PE = nc.alloc_semaphore("S_PE")
    S_ACT = nc.alloc_semaphore("S_ACT")
    S_DVE = nc.alloc_semaphore("S_DVE")
    S_POOL = nc.alloc_semaphore("S_POOL")
    S_OUT = nc.alloc_semaphore("S_OUT")

    cnt = {"pe": 0, "act": 0, "dve": 0, "pool": 0, "in": 0, "out": 0}

    # ---- input DMAs: layer-1 weights first, then x chunk by chunk, then
    # layer-2 weights, so the prologue only gates on what layer 1 needs ----
    def emit_weight_dmas(cells_subset):
        for cell in cells_subset:
            nc.sync.dma_start(out=whh_s[cell].ap()[:, :],
                              in_=whh_d[cell].ap()[:, :]).then_inc(S_IN, 16)
            nc.sync.dma_start(out=bias_s[cell].ap()[:, :],
                              in_=bias_d[cell].ap()[:, :]).then_inc(S_IN, 16)
            cnt["in"] += 32
            for q, dd in enumerate(wih_d[cell]):
                nc.sync.dma_start(out=wih_s[cell][q].ap()[:, :],
                                  in_=dd.ap()[:, :]).then_inc(S_IN, 16)
                cnt["in"] += 16

    emit_weight_dmas(["a1", "b1"])
    ndma = 8
    chunk = TB // ndma
    in_after_chunk = []
    for i in range(ndma):
        for src_t, dst_t in ((xT_d, xT), (xTr_d, xTr)):
            nc.sync.dma_start(out=dst_t.ap()[:, i * chunk:(i + 1) * chunk],
                              in_=src_t.ap()[:, i * chunk:(i + 1) * chunk]
                              ).then_inc(S_IN, 16)
            cnt["in"] += 16
        in_after_chunk.append(cnt["in"])
    emit_weight_dmas(["a2", "b2"])
    in_total = cnt["in"]

    nc.vector.memset(ones_row.ap()[:, :], 1.0).then_inc(S_DVE, 1)
    nc.vector.memset(c_init.ap()[:, :], 0.0).then_inc(S_DVE, 1)
    cnt["dve"] += 2
    # gate Pool on c_init zero (m1 of step 0 reads it)
    nc.gpsimd.wait_ge(S_DVE, cnt["dve"])

    W = CH * BL  # columns per gate region in a bank / x columns per window

    def chunk_for_cols(last_col):
        return min(ndma - 1, last_col // chunk)

    # PE prologue gates: weights + x chunks for windows 0-1 + ones_row ready
    last_ck = chunk_for_cols(2 * W - 1)
    nc.tensor.wait_ge(S_IN, in_after_chunk[last_ck])
    nc.tensor.wait_ge(S_DVE, cnt["dve"])

    act_after_win = {}   # (lay, ci, window) -> S_ACT count after last sigma

    for lay in (1, 2):
        cells = (f"a{lay}", f"b{lay}")
        if lay == 1:
            rhs_src = {0: [(xT, 0)], 1: [(xTr, 0)]}
        else:
            rhs_src = {0: [(h_t[(1, 0)], 0), (h1bt, 0)],
                       1: [(h1ar, 0), (h_t[(1, 1)], 0)]}

        def precompute_mms(ci, j, inc_pe=False):
            """Emission closures for window j's precompute, so the matmuls
            can be dribbled a few per step (a single burst would head-of-line
            block the recurrence matmuls behind it on the FIFO PE queue)."""
            ps = pb[ci][j % 2].ap()
            war = act_after_win.pop((lay, ci, j), None)
            out = []
            for g in range(G):
                reg = g * W
                for q, (src, base) in enumerate(rhs_src[ci]):
                    def mk(g=g, q=q, src=src, base=base, reg=reg,
                           first=not out, war=war):
                        mm = nc.tensor.matmul(
                            ps[:, reg:reg + W],
                            wih_s[cells[ci]][q].ap()[:, g * H:(g + 1) * H],
                            src.ap()[:, base + j * W:base + (j + 1) * W],
                            start=first, stop=False)
                        if first and war is not None:
                            mm._wait_ge(S_ACT, war)
                    out.append(mk)
            for g in range(G):
                def mk(g=g, last=(g == G - 1)):
                    mm = nc.tensor.matmul(
                        ps[:, g * W:g * W + W],
                        bias_s[cells[ci]].ap()[:, g * H:(g + 1) * H],
                        ones_row.ap()[:, :],
                        start=False, stop=last)
                    if last and inc_pe:
                        mm.then_inc(S_PE, 1)
                        cnt["pe"] += 1
                out.append(mk)
            return out

        def precompute(ci, j, inc_pe=False):
            for mk in precompute_mms(ci, j, inc_pe):
                mk()
            return cnt["pe"]

        if lay == 2:
            # layer boundary: h1 / h1bt / h1ar fully written
            nc.tensor.wait_ge(S_DVE, cnt["dve"])
            nc.tensor.wait_ge(S_POOL, cnt["pool"])
            nc.tensor.wait_ge(S_IN, in_total)  # layer-2 weights loaded
        pe_cnt = {}
        pending = []
        for ci in (0, 1):
            precompute(ci, 0, inc_pe=True)
            pe_cnt[ci] = cnt["pe"]
            # window 1 is dribbled into the first steps instead of primed
            # as one blob (it is only read from step CH on)
            pending.extend(precompute_mms(ci, 1))
        npc = 2 if lay == 1 else 3  # precompute matmuls emitted per step
        h_cnt = {0: None, 1: None}
        c_cnt = {}
        c_prev = {0: c_init.ap()[:, :], 1: c_init.ap()[:, :]}
        for k in range(t_len):
            j, slot = k // CH, k % CH
            # --- PE: recurrence matmuls, chain A then B ---
            if k > 0:
                for ci in (0, 1):
                    ps = pb[ci][j % 2].ap()
                    hprev = h_t[(lay, ci)].ap()[:, (k - 1) * BL:k * BL]
                    first = True
                    for g in range(G):
                        off = g * W + slot * BL
                        mm = nc.tensor.matmul(
                            ps[:, off:off + BL],
                            whh_s[cells[ci]].ap()[:, g * H:(g + 1) * H],
                            hprev, start=False, stop=False,
                            skip_group_check=True)
                        if first:
                            mm._wait_ge(S_DVE, h_cnt[ci])
                            first = False
                    mm.then_inc(S_PE, 1)
                    cnt["pe"] += 1
                    pe_cnt[ci] = cnt["pe"]
            # --- ACT: sigmas (A then B) ---
            sig_cnt = {}
            for ci in (0, 1):
                ps = pb[ci][j % 2].ap()
                s = s_r[ci][k % 2].ap()
                ps_view = ps[:, :].rearrange(
                    "p (g t b) -> p g t b", g=G, t=CH)[:, :, slot, :]
                s_view = s[:, :].rearrange("p (g b) -> p g b", g=G)
                sig = nc.scalar.activation(s_view, ps_view, AF.Sigmoid)
                sig._wait_ge(S_PE, pe_cnt[ci])
                sig.then_inc(S_ACT, 1)
                cnt["act"] += 1
                sig_cnt[ci] = cnt["act"]
                if slot == CH - 1:
                    act_after_win[(lay, ci, j + 2)] = cnt["act"]
            # --- DVE m2 + Pool m1, then DVE c (A then B) ---
            m1_cnt = {}
            for ci in (0, 1):
                s = s_r[ci][k % 2].ap()
                si, sf = s[:, 0:BL], s[:, BL:2 * BL]
                s2g = s[:, 2 * BL:3 * BL]
                m2 = m2_r[ci][k % 2].ap()
                op = nc.vector.scalar_tensor_tensor(
                    m2[:, :], s2g, 0.5, si, ALU.subtract, ALU.mult)
                op._wait_ge(S_ACT, sig_cnt[ci])
                m1 = m1_r[ci][k % 2].ap()
                m1op = nc.gpsimd.tensor_tensor(m1[:, :], sf, c_prev[ci],
                                               ALU.mult)
                m1op._wait_ge(S_ACT, sig_cnt[ci])
                m1op.then_inc(S_POOL, 1)
                cnt["pool"] += 1
                c = c_r[ci][k % 2].ap()
                cop = nc.vector.scalar_tensor_tensor(
                    c[:, :], m2[:, :], 2.0, m1[:, :], ALU.mult, ALU.add)
                cop._wait_ge(S_POOL, cnt["pool"])
                cop.then_inc(S_DVE, 1)
                cnt["dve"] += 1
                c_cnt[ci] = cnt["dve"]
                c_prev[ci] = c[:, :]
            # --- ACT tanh (A then B) ---
            th_cnt = {}
            for ci in (0, 1):
                th = th_r[ci][k % 2].ap()
                ti = nc.scalar.activation(th[:, :], c_r[ci][k % 2].ap()[:, :],
                                          AF.Tanh)
                ti._wait_ge(S_DVE, c_cnt[ci])
                ti.then_inc(S_ACT, 1)
                cnt["act"] += 1
                th_cnt[ci] = cnt["act"]
            # --- DVE h-mult (A then B) ---
            for ci in (0, 1):
                s = s_r[ci][k % 2].ap()
                so = s[:, 3 * BL:4 * BL]
                hop = nc.vector.tensor_tensor(
                    h_t[(lay, ci)].ap()[:, k * BL:(k + 1) * BL],
                    so, th_r[ci][k % 2].ap()[:, :], ALU.mult)
                hop._wait_ge(S_ACT, th_cnt[ci])
                hop.then_inc(S_DVE, 1)
                cnt["dve"] += 1
                h_cnt[ci] = cnt["dve"]
            # --- Pool copies for layer-2 precompute layouts ---
            if lay == 1:
                tau = t_len - 1 - k
                cp1 = nc.gpsimd.tensor_copy(
                    h1bt.ap()[:, tau * BL:(tau + 1) * BL],
                    h_t[(1, 1)].ap()[:, k * BL:(k + 1) * BL])
                cp1._wait_ge(S_DVE, h_cnt[1])
                cp1.then_inc(S_POOL, 1)
                cp2 = nc.gpsimd.tensor_copy(
                    h1ar.ap()[:, tau * BL:(tau + 1) * BL],
                    h_t[(1, 0)].ap()[:, k * BL:(k + 1) * BL])
                cp2._wait_ge(S_DVE, h_cnt[0])
                cp2.then_inc(S_POOL, 1)
                cnt["pool"] += 2
            # --- precompute next window: dribble a few matmuls per step ---
            if slot == 3 and 2 <= j + 1 < nw:
                if lay == 1:
                    ck = chunk_for_cols((j + 2) * W - 1)
                    if ck > last_ck:
                        # gate PE on the x chunks window j+1 reads
                        nc.tensor.wait_ge(S_IN, in_after_chunk[ck])
                        last_ck = ck
                for ci in (0, 1):
                    pending.extend(precompute_mms(ci, j + 1))
            npop = min(len(pending), npc)
            for mk in pending[:npop]:
                mk()
            del pending[:npop]
            # --- stream output ---
            if lay == 2:
                ock = t_len // 8
                if (k + 1) % ock == 0:
                    lo = (k + 1 - ock) * BL
                    hi = (k + 1) * BL
                    for ci in (0, 1):
                        dma = nc.sync.dma_start(
                            out=o2_d.ap()[:, ci * TB + lo:ci * TB + hi],
                            in_=h_t[(2, ci)].ap()[:, lo:hi])
                        dma._wait_ge(S_DVE, h_cnt[ci])
                        dma.then_inc(S_OUT, 16)
                        cnt["out"] += 16

    nc.sync.wait_ge(S_OUT, cnt["out"])
    nc.all_engine_barrier()
    nc.compile()
    return nc


def _prep_weights(Wih, Whh, bih, bhh):
    """Host-side weight massaging: transpose, gate-scale (g-gate x2), bf16."""
    gscale = np.array([1.0, 1.0, 2.0, 1.0], np.float32)
    fourh, ind = Wih.shape
    wihT = np.ascontiguousarray(Wih.T).astype(np.float32)      # [in, 4H]
    whhT = np.ascontiguousarray(Whh.T).astype(np.float32)      # [H, 4H]
    bias = (bih + bhh).astype(np.float32)                      # [4H]
    for g in range(G):
        sl = slice(g * H, (g + 1) * H)
        wihT[:, sl] *= gscale[g]
        whhT[:, sl] *= gscale[g]
        bias[sl] *= gscale[g]
    nq = ind // H
    wih_chunks = [np.ascontiguousarray(wihT[q * H:(q + 1) * H]).astype(BF16_NP)
                  for q in range(nq)]
    bias_row = bias.reshape(1, G * H).astype(BF16_NP)           # [1, 4H]
    return wih_chunks, whhT.astype(BF16_NP), bias_row


def core_xT(xs, t_len):
    """Per-core input layouts: xs [BL, T, D] -> xT, xTr both [D, T*BL]."""
    fw = xs.transpose(2, 1, 0).reshape(D, t_len * BL)
    bw = xs[:, ::-1, :].transpose(2, 1, 0).reshape(D, t_len * BL)
    return (np.ascontiguousarray(fw).astype(BF16_NP),
            np.ascontiguousarray(bw).astype(BF16_NP))


def core_gather(res_c, t_len):
    """Per-core output: o2 [H, 2*T*BL] (bw half step-indexed) -> [BL,T,2H]."""
    o = np.asarray(res_c["o2"]).astype(np.float32)
    TB = t_len * BL
    out = np.empty((BL, t_len, 2 * H), np.float32)
    fw = o[:, :TB].reshape(H, t_len, BL)
    bw = o[:, TB:].reshape(H, t_len, BL)[:, ::-1, :]  # step k -> time T-1-k
    out[:, :, :H] = fw.transpose(2, 1, 0)
    out[:, :, H:] = bw.transpose(2, 1, 0)
    return out


_PROG_CACHE = {}


def prepare_in_maps(x, kw):
    x = np.asarray(x, np.float32)
    t_len = x.shape[1]
    cells = {"a1": (kw["Wih_fw1"], kw["Whh_fw1"], kw["bih_fw1"], kw["bhh_fw1"]),
             "b1": (kw["Wih_bw1"], kw["Whh_bw1"], kw["bih_bw1"], kw["bhh_bw1"]),
             "a2": (kw["Wih_fw2"], kw["Whh_fw2"], kw["bih_fw2"], kw["bhh_fw2"]),
             "b2": (kw["Wih_bw2"], kw["Whh_bw2"], kw["bih_bw2"], kw["bhh_bw2"])}
    wmaps = {}
    for cell, (Wih, Whh, bih, bhh) in cells.items():
        wih_chunks, whhT, bias_row = _prep_weights(
            np.asarray(Wih, np.float32), np.asarray(Whh, np.float32),
            np.asarray(bih, np.float32), np.asarray(bhh, np.float32))
        wmaps[f"whhT_{cell}"] = whhT
        wmaps[f"bias_{cell}"] = bias_row
        for q, wc in enumerate(wih_chunks):
            wmaps[f"wihT_{cell}_{q}"] = wc

    core_ids = list(range(NCORES))
    in_maps = []
    for c in core_ids:
        fw, bw = core_xT(x[c * BL:(c + 1) * BL], t_len)
        m = {"xT": fw, "xTr": bw}
        m.update(wmaps)
        in_maps.append(m)
    return in_maps, core_ids


def kernel(x, lengths, **kw):
    x = np.asarray(x, np.float32)
    t_len = x.shape[1]
    in_maps, core_ids = prepare_in_maps(x, kw)
    if t_len not in _PROG_CACHE:
        _PROG_CACHE[t_len] = build_program(t_len)
    nc = _PROG_CACHE[t_len]
    return _execute(nc, in_maps, core_ids, t_len)[0]


def _execute(nc, in_maps, core_ids, t_len, **run_kwargs):
    r = run_bass_kernel_spmd(nc, in_maps, core_ids, **run_kwargs)
    out = np.empty((B, t_len, 2 * H), np.float32)
    for c in core_ids:
        out[c * BL:(c + 1) * BL] = core_gather(r.results[c], t_len)
    return out, r
